# revision 8
# baseline (speedup 1.0000x reference)
"""AKConv TRN2 kernel: 8-core data-parallel over batch.

Sample-major gather architecture: dma_gather(transpose=False) puts each
sample (4 corners x 128ch) on one partition; bilinear combine on DVE via
stride-0 broadcast weights; PE transposes chunks back to channel-major,
accumulates the BN gram + row sums off the same stationary, and runs the
1x1 conv for both outc halves inline. Tail: allreduce -> BN coeffs ->
SiLU -> output DMA.
"""
import sys
sys.path.insert(0, "/opt/trn_rl_repo")
import math
import numpy as np
import ml_dtypes
import bass_rust
import concourse.bass as bass
import concourse.tile as tile
from concourse import bacc, mybir
from concourse.bass_utils import run_bass_kernel_spmd

F32 = mybir.dt.float32
BF16 = mybir.dt.bfloat16
I16 = mybir.dt.int16
AF = mybir.ActivationFunctionType
OP = mybir.AluOpType

B, INC, H, W = 8, 128, 64, 64
OUTC, N = 256, 9
S = H * W                      # 4096 pixels per core
NS = N * S                     # 36864 samples per core
S_TOT = float(B * NS)          # BN sample count
EPS = 1e-5
N_CORES = 8

GH = 2048                      # samples per dma_gather
NG = NS // GH                  # 18 gathers
NCHUNK = GH // 128             # 16 chunks of 128 samples per gather


def _ap_raw(ap, offset, dims):
    a = ap.copy()
    a.offset = offset
    a.ap = bass_rust.VecI64Pair(dims)
    return a


def build(stage=3):
    nc = bacc.Bacc("TRN2", target_bir_lowering=False, debug=False,
                   num_devices=N_CORES)
    xpad_d = nc.dram_tensor("xpad", [INC, 66 * 66], F32, kind="ExternalInput")
    x2_d = nc.dram_tensor("x2", [S, 2 * INC], BF16, kind="ExternalInput")
    pwt_d = nc.dram_tensor("pwt", [INC, 9, 2 * N], F32, kind="ExternalInput")
    base4_d = nc.dram_tensor("base4", [128, 32, 2 * N], F32, kind="ExternalInput")
    cwt_d = nc.dram_tensor("cwt", [INC, OUTC], F32, kind="ExternalInput")
    gb_d = nc.dram_tensor("gb", [1, 2 * OUTC], F32, kind="ExternalInput")
    id18_d = nc.dram_tensor("id18", [18, 18], F32, kind="ExternalInput")
    id128_d = nc.dram_tensor("id128", [128, 128], BF16, kind="ExternalInput")
    out_d = nc.dram_tensor("out", [OUTC, NS], BF16, kind="ExternalOutput")

    idx_dram = nc.dram_tensor("idx_scratch", [1, NS], I16, kind="Internal")
    w_dram = nc.dram_tensor("w_scratch", [4, NS], BF16, kind="Internal")
    ab_dram = nc.dram_tensor("ab_scratch", [OUTC, 2], F32, kind="Internal")

    with tile.TileContext(nc) as tc:
        with tc.tile_pool(name="persist", bufs=1) as pp, \
             tc.tile_pool(name="work", bufs=1) as wp, \
             tc.tile_pool(name="dram", bufs=1, space="DRAM") as dp:

            # ---------- loads ----------
            pwt = pp.tile([INC, 9, 2 * N], F32)
            nc.sync.dma_start(pwt[:], pwt_d[:])
            base4 = pp.tile([128, 32, 2 * N], F32)
            nc.sync.dma_start(base4[:], base4_d[:])
            cwt = pp.tile([INC, OUTC], F32)
            nc.sync.dma_start(cwt[:], cwt_d[:])
            gb = pp.tile([1, 2 * OUTC], F32)
            nc.sync.dma_start(gb[:], gb_d[:])
            id128 = pp.tile([128, 128], BF16)
            nc.sync.dma_start(id128[:], id128_d[:])
            cwt_b = pp.tile([INC, OUTC], BF16)
            nc.vector.tensor_copy(cwt_b[:], cwt[:])
            ones_b = pp.tile([128, 1], BF16)
            nc.vector.memset(ones_b[:], 1.0)
            ones_f = pp.tile([128, 1], F32)
            nc.vector.memset(ones_f[:], 1.0)
            warm_in = dp.tile([1, 4], F32)
            warm_out = dp.tile([1, 4], F32, addr_space="Shared")
            if stage >= 3:
                nc.sync.dma_start(warm_in[:], gb_d[:, 0:4])
                nc.gpsimd.collective_compute(
                    "AllReduce", OP.add,
                    replica_groups=[list(range(N_CORES))],
                    ins=[warm_in[:].opt()],
                    outs=[warm_out[:].opt()])
            posT = pp.tile([128, 32, 2 * N], F32)

            # ---------- p_conv: offsets (18, S) then transpose ----------
            id18 = pp.tile([18, 18], F32)
            nc.sync.dma_start(id18[:], id18_d[:])
            with tc.tile_pool(name="xpadp", bufs=1) as xp, \
                 tc.tile_pool(name="psum_pc", bufs=4, space="PSUM") as pcp:
                xpad = xp.tile([INC, 66 * 66], F32)
                nc.sync.dma_start(xpad[:], xpad_d[:])
                xpad_ap = xpad[:]
                pstride = xpad_ap.ap[0][0]
                base_off = xpad_ap.offset
                offs = xp.tile([18, S], F32)
                for c8 in range(8):
                    acc = pcp.tile([18, 512], F32, tag="pconv")
                    for tap in range(9):
                        dh, dw = tap // 3, tap % 3
                        mov = _ap_raw(xpad_ap,
                                      base_off + (c8 * 8 + dh) * 66 + dw,
                                      [(pstride, 128), (66, 8), (1, 64)])
                        nc.tensor.matmul(acc[:], pwt[:, tap, :], mov,
                                         start=(tap == 0), stop=(tap == 8))
                    nc.vector.tensor_copy(offs[:, c8 * 512:(c8 + 1) * 512],
                                          acc[:])
                for t in range(32):
                    tp = pcp.tile([128, 18], F32, tag="tpose")
                    nc.tensor.transpose(tp[:], offs[:, t * 128:(t + 1) * 128],
                                        id18[:])
                    nc.vector.tensor_copy(posT[:, t, :], tp[:])

            # ---------- positions / weights / indices ----------
            def ts(out, in_, s1, o1, s2=None, o2=None):
                if s2 is None:
                    nc.vector.tensor_scalar(out, in_, s1, None, op0=o1)
                else:
                    nc.vector.tensor_scalar(out, in_, s1, s2, op0=o1, op1=o2)

            _slab_ctr = [0]

            def slab():
                _slab_ctr[0] += 1
                return wpp.tile([128, 32, 2 * N], F32,
                                name=f"slab{_slab_ctr[0]}",
                                tag=f"slab{_slab_ctr[0]}")

            _wpp_cm = tc.tile_pool(name="wrapp", bufs=1)
            wpp = _wpp_cm.__enter__()
            p4 = base4                       # reuse base4 storage for p4
            nc.vector.tensor_add(p4[:], posT[:], base4[:])
            pc4 = slab()
            ts(pc4[:], p4[:], 4.0, OP.max, 67.0, OP.min)
            i32 = wpp.tile([128, 32, 2 * N], mybir.dt.int32)
            nc.vector.tensor_copy(i32[:], p4[:])
            mfr = slab()
            nc.vector.tensor_copy(mfr[:], i32[:])
            f4 = slab()
            nc.vector.tensor_tensor(f4[:], p4[:], mfr[:], op=OP.is_lt)
            nc.vector.tensor_tensor(f4[:], mfr[:], f4[:], op=OP.subtract)
            qlt = mfr                        # reuse
            ts(qlt[:], f4[:], 4.0, OP.max, 67.0, OP.min)
            qrb = slab()
            ts(qrb[:], f4[:], 1.0, OP.add, 4.0, OP.max)
            ts(qrb[:], qrb[:], 67.0, OP.min)
            g04 = slab()
            ts(g04[:], f4[:], 4.0, OP.max, 66.0, OP.min)
            ax = slab()
            nc.vector.tensor_tensor(ax[:], qlt[:], pc4[:], op=OP.subtract)
            ts(ax[:], ax[:], 1.0, OP.add)
            bx = qlt                         # reuse (qlt dead)
            nc.vector.tensor_tensor(bx[:], pc4[:], qrb[:], op=OP.subtract)
            ts(bx[:], bx[:], 1.0, OP.add)
            c1 = pc4                         # reuse (pc4 dead)
            ts(c1[:], f4[:], 66.0, OP.is_le)
            c2 = qrb                         # reuse (qrb dead)
            ts(c2[:], f4[:], 3.0, OP.is_le)
            w0 = f4                          # reuse (f4 dead)
            nc.vector.tensor_tensor(w0[:], ax[:], c1[:], op=OP.mult)
            t0 = slab()
            nc.vector.tensor_tensor(t0[:], bx[:], c2[:], op=OP.mult)
            nc.vector.tensor_add(w0[:], w0[:], t0[:])
            w1 = t0                          # reuse
            nc.vector.tensor_add(w1[:], ax[:], bx[:])
            nc.vector.tensor_tensor(w1[:], w1[:], w0[:], op=OP.subtract)

            # corner weights -> wslab[p, t, j, n] bf16; j: 0=x2 row order
            # (lt, lb, rt, rb) = (w0x*w0y, w1x*w0y, w0x*w1y, w1x*w1y)
            wslab = wpp.tile([128, 32, 4, 9], BF16)
            for j, (wx, wy) in enumerate([(w0, w0), (w1, w0), (w0, w1), (w1, w1)]):
                nc.vector.tensor_tensor(wslab[:, :, j, :], wx[:, :, 0:N],
                                        wy[:, :, N:2 * N], op=OP.mult)

            # idx = g04x*64 + g04y - 260 (4-space shift removal)
            idxf = ax                        # reuse (ax dead), use x-half
            ts(idxf[:, :, 0:N], g04[:, :, 0:N], 64.0, OP.mult, 260.0,
               OP.subtract)
            nc.vector.tensor_add(idxf[:, :, 0:N], idxf[:, :, 0:N],
                                 g04[:, :, N:2 * N])
            idx16 = wpp.tile([128, 32, 9], I16)
            nc.vector.tensor_copy(idx16[:], idxf[:, :, 0:N])

            # route to k-linear DRAM order, k = 9*(128t + p) + n
            nc.sync.dma_start(
                _ap_raw(idx_dram[:], 0, [(9, 128), (1152, 32), (1, 9)]),
                idx16[:])
            for j in range(4):
                nc.sync.dma_start(
                    _ap_raw(w_dram[:], j * NS,
                            [(9, 128), (1152, 32), (1, 9)]),
                    wslab[:, :, j, :])
            _wpp_cm.__exit__(None, None, None)

            # gather source AP over x2: overlapping 2-row windows
            x2_src = _ap_raw(x2_d[:], 0, [(2 * INC, S - 1), (1, 4 * INC)])

            # ---------- main loop: gather / combine / PE / y ----------
            y_raw0 = pp.tile([128, NS], BF16)
            y_raw1 = pp.tile([128, NS], BF16)
            chunk_ctr = [0]
            TOT_CHUNKS = NG * NCHUNK
            with tc.tile_pool(name="gather", bufs=2) as gp, \
                 tc.tile_pool(name="xo", bufs=2) as xop, \
                 tc.tile_pool(name="xcmp", bufs=3) as xcp, \
                 tc.tile_pool(name="psum_g", bufs=1, space="PSUM") as gpp, \
                 tc.tile_pool(name="psum_tp", bufs=2, space="PSUM") as tpp, \
                 tc.tile_pool(name="psum_y", bufs=2, space="PSUM") as typ:
                gm1 = gpp.tile([128, 129], F32, tag="gram")
                gpsum = gm1[:, 0:128]
                m1psum = gm1[:, 128:129]
                for g in range(NG):
                    idxw = gp.tile([128, GH // 16], I16, tag="idxw")
                    idx_src = _ap_raw(idx_dram[:], g * GH,
                                      [(1, 16), (16, GH // 16)])
                    for v in range(8):
                        nc.sync.dma_start(idxw[16 * v:16 * (v + 1)], idx_src)
                    w4 = gp.tile([128, 4, NCHUNK], BF16, tag="w4")
                    for j in range(4):
                        nc.sync.dma_start(
                            w4[:, j, :],
                            _ap_raw(w_dram[:], j * NS + g * GH,
                                    [(1, 128), (128, NCHUNK)]))
                    dst = gp.tile([128, NCHUNK, 512], BF16, tag="gdst")
                    nc.gpsimd.dma_gather(
                        dst[:], x2_src, idxw[:], GH, GH, 4 * INC,
                        elem_step=2 * INC, transpose=False,
                        single_packet=False)

                    # bilinear combine on DVE (stride-0 broadcast weights)
                    dd = dst[:]
                    doff, dstr = dd.offset, dd.ap[0][0]
                    ww = w4[:]
                    woff, wstr = ww.offset, ww.ap[0][0]

                    def ds(j):
                        return _ap_raw(dd, doff + j * 128,
                                       [(dstr, 128), (512, NCHUNK), (1, 128)])

                    def wb(j):
                        return _ap_raw(ww, woff + j * NCHUNK,
                                       [(wstr, 128), (1, NCHUNK), (0, 128)])

                    xo = xop.tile([128, NCHUNK, 128], BF16, tag="xo")
                    nc.vector.tensor_tensor(ds(0), ds(0), wb(0), op=OP.mult)
                    nc.vector.tensor_tensor(ds(1), ds(1), wb(1), op=OP.mult)
                    nc.vector.tensor_tensor(ds(0), ds(0), ds(1), op=OP.add)
                    nc.vector.tensor_tensor(ds(2), ds(2), wb(2), op=OP.mult)
                    nc.vector.tensor_tensor(ds(3), ds(3), wb(3), op=OP.mult)
                    nc.vector.tensor_tensor(ds(2), ds(2), ds(3), op=OP.add)
                    nc.vector.tensor_tensor(xo[:], ds(0), ds(2), op=OP.add)

                    # per 4-chunk group: transpose+gram+m1, then y matmuls
                    for grp in range(NCHUNK // 4):
                        xcm = xcp.tile([128, 4, 128], BF16, tag="xcm")
                        for q in range(4):
                            i = grp * 4 + q
                            ci = chunk_ctr[0]
                            chunk_ctr[0] += 1
                            tp = tpp.tile([128, 128], BF16, tag="tp")
                            nc.tensor.transpose(tp[:], xo[:, i, :], id128[:])
                            nc.tensor.matmul(gpsum, xo[:, i, :],
                                             xo[:, i, :],
                                             start=(ci == 0),
                                             stop=(ci == TOT_CHUNKS - 1),
                                             skip_group_check=True)
                            nc.tensor.matmul(m1psum, xo[:, i, :], ones_b[:],
                                             start=(ci == 0),
                                             stop=(ci == TOT_CHUNKS - 1),
                                             skip_group_check=True)
                            nc.scalar.activation(xcm[:, q, :], tp[:], AF.Copy)
                        xmov = xcm[:].rearrange("p a b -> p (a b)")
                        col = g * GH + grp * 512
                        y0p = typ.tile([128, 512], F32, tag="y0")
                        nc.tensor.matmul(y0p[:], cwt_b[:, 0:128], xmov,
                                         start=True, stop=True)
                        y1p = typ.tile([128, 512], F32, tag="y1")
                        nc.tensor.matmul(y1p[:], cwt_b[:, 128:256], xmov,
                                         start=True, stop=True)
                        nc.scalar.activation(y_raw0[:, col:col + 512], y0p[:],
                                             AF.Copy)
                        nc.vector.tensor_copy(y_raw1[:, col:col + 512], y1p[:])

                g_sb = wp.tile([128, 129], F32)
                nc.vector.tensor_copy(g_sb[:], gm1[:])

            # ---------- allreduce + BN coefficients ----------
            gsum = wp.tile([128, 129], F32)
            if stage >= 3:
                bounce_in = dp.tile([128, 129], F32)
                bounce_out = dp.tile([128, 129], F32, addr_space="Shared")
                nc.sync.dma_start(bounce_in[:], g_sb[:])
                nc.gpsimd.collective_compute(
                    "AllReduce", OP.add,
                    replica_groups=[list(range(N_CORES))],
                    ins=[bounce_in[:].opt()],
                    outs=[bounce_out[:].opt()])
                nc.sync.dma_start(gsum[:], bounce_out[:])
            else:
                nc.vector.tensor_scalar(gsum[:], g_sb[:], 8.0, None,
                                        op0=OP.mult)

            with tc.tile_pool(name="psum_s", bufs=1, space="PSUM") as sp:
                t1p = sp.tile([128, OUTC], F32)
                nc.tensor.matmul(t1p[:], gsum[:, 0:128], cwt[:],
                                 start=True, stop=True)
                m2 = wp.tile([128, OUTC], F32)
                nc.vector.tensor_tensor(m2[:], cwt[:], t1p[:], op=OP.mult)
                dvp = sp.tile([1, OUTC], F32)
                nc.tensor.matmul(dvp[:], ones_f[:], m2[:], start=True,
                                 stop=True)
                m1yp = sp.tile([1, OUTC], F32)
                nc.tensor.matmul(m1yp[:], gsum[:, 128:129], cwt[:],
                                 start=True, stop=True)

                meanv = wp.tile([1, OUTC], F32)
                ts(meanv[:], m1yp[:], 1.0 / S_TOT, OP.mult)
                varv = wp.tile([1, OUTC], F32)
                ts(varv[:], dvp[:], 1.0 / S_TOT, OP.mult)
                msq = wp.tile([1, OUTC], F32)
                nc.vector.tensor_tensor(msq[:], meanv[:], meanv[:], op=OP.mult)
                nc.vector.tensor_tensor(varv[:], varv[:], msq[:],
                                        op=OP.subtract)
                ts(varv[:], varv[:], EPS, OP.add)
                sd = wp.tile([1, OUTC], F32)
                nc.scalar.activation(sd[:], varv[:], AF.Sqrt)
                rsd = wp.tile([1, OUTC], F32)
                nc.vector.reciprocal(rsd[:], sd[:])
                a_v = wp.tile([1, OUTC], F32)
                nc.vector.tensor_tensor(a_v[:], rsd[:], gb[:, 0:OUTC],
                                        op=OP.mult)
                b_v = wp.tile([1, OUTC], F32)
                nc.vector.tensor_tensor(b_v[:], meanv[:], a_v[:], op=OP.mult)
                nc.vector.tensor_tensor(b_v[:], gb[:, OUTC:2 * OUTC], b_v[:],
                                        op=OP.subtract)

            nc.sync.dma_start(_ap_raw(ab_dram[:], 0, [(2, OUTC)]), a_v[:])
            nc.sync.dma_start(_ap_raw(ab_dram[:], 1, [(2, OUTC)]), b_v[:])
            ab = pp.tile([128, 2, 2], F32)
            nc.sync.dma_start(
                ab[:], _ap_raw(ab_dram[:], 0, [(2, 128), (256, 2), (1, 2)]))

            # ---------- silu epilogue ----------
            YB = 4096
            with tc.tile_pool(name="ybuf", bufs=3) as yb:
                for half, ysrc in ((0, y_raw0), (1, y_raw1)):
                    for blk in range(NS // YB):
                        ybuf = yb.tile([128, YB], BF16, tag="yb")
                        nc.scalar.activation(
                            ybuf[:], ysrc[:, blk * YB:(blk + 1) * YB],
                            AF.Silu, scale=ab[:, half, 0:1],
                            bias=ab[:, half, 1:2])
                        out_ap = _ap_raw(
                            out_d[:], half * 128 * NS + blk * YB,
                            [(NS, 128), (1, YB)])
                        nc.sync.dma_start(out_ap, ybuf[:])

    nc.compile()
    return nc


def prep_inputs(x, pw, pb, cw, gamma, beta):
    x = np.asarray(x, np.float32)
    pw = np.asarray(pw, np.float32)
    pb = np.asarray(pb, np.float32)
    cw = np.asarray(cw, np.float32)
    gamma = np.asarray(gamma, np.float32)
    beta = np.asarray(beta, np.float32)

    pwt = np.ascontiguousarray(
        pw.reshape(2 * N, INC, 9).transpose(1, 2, 0))      # (128, 9, 18)

    angles = np.linspace(0.0, 2.0 * math.pi, N + 1, dtype=np.float64)[:-1]
    pn = np.concatenate([np.cos(angles), np.sin(angles)]).astype(np.float32)
    p_idx = np.arange(128)
    t_idx = np.arange(32)
    hh = (2 * t_idx[None, :] + (p_idx[:, None] >= 64)).astype(np.float32)
    ww = np.broadcast_to((p_idx % 64).astype(np.float32)[:, None], (128, 32))
    base4 = np.zeros((128, 32, 2 * N), np.float32)
    base4[:, :, 0:N] = hh[:, :, None] + (pn[0:N] + pb[0:N])[None, None, :] + 4.0
    base4[:, :, N:] = ww[:, :, None] + (pn[N:] + pb[N:])[None, None, :] + 4.0

    cwt = np.ascontiguousarray(cw[:, :, 0, 0].T)           # (128, 256)
    gb = np.concatenate([gamma, beta])[None, :]            # (1, 512)
    id128 = np.eye(128, dtype=np.float32).astype(ml_dtypes.bfloat16)

    in_maps = []
    for b in range(B):
        xb = x[b].reshape(INC, S)
        xpad = np.zeros((INC, 66, 66), np.float32)
        xpad[:, 1:65, 1:65] = x[b]
        xT = np.ascontiguousarray(xb.T).astype(ml_dtypes.bfloat16)  # (4096, 128)
        x2 = np.zeros((S, 2 * INC), ml_dtypes.bfloat16)
        x2[:, 0:INC] = xT
        x2[:S - 64, INC:] = xT[64:]
        in_maps.append(dict(
            xpad=np.ascontiguousarray(xpad.reshape(INC, 66 * 66)), x2=x2,
            pwt=pwt, base4=base4, cwt=cwt, gb=gb,
            id18=np.eye(18, dtype=np.float32), id128=id128))
    return in_maps


_NC_CACHE = {}


def kernel(x, pw, pb, cw, gamma, beta):
    if "nc" not in _NC_CACHE:
        _NC_CACHE["nc"] = build()
    nc = _NC_CACHE["nc"]
    in_maps = prep_inputs(x, pw, pb, cw, gamma, beta)
    res = run_bass_kernel_spmd(nc, in_maps, core_ids=list(range(N_CORES)))
    out = np.stack([
        np.asarray(res.results[b]["out"]).astype(np.float32).reshape(
            OUTC, H, W * N)
        for b in range(B)])
    return out


# revision 13
# speedup vs baseline: 1.1875x; 1.1875x over previous
"""AKConv TRN2 kernel: 8-core data-parallel over batch.

Sample-major gather architecture: dma_gather(transpose=False) puts each
sample (4 corners x 128ch) on one partition; bilinear combine on DVE via
stride-0 broadcast weights; PE transposes chunks back to channel-major,
accumulates the BN gram + row sums off the same stationary, and runs the
1x1 conv for both outc halves inline. Tail: allreduce -> BN coeffs ->
SiLU -> output DMA.
"""
import sys
sys.path.insert(0, "/opt/trn_rl_repo")
import math
import numpy as np
import ml_dtypes
import bass_rust
import concourse.bass as bass
import concourse.tile as tile
from concourse import bacc, mybir
from concourse.bass_utils import run_bass_kernel_spmd

F32 = mybir.dt.float32
BF16 = mybir.dt.bfloat16
I16 = mybir.dt.int16
AF = mybir.ActivationFunctionType
OP = mybir.AluOpType

B, INC, H, W = 8, 128, 64, 64
OUTC, N = 256, 9
S = H * W                      # 4096 pixels per core
NS = N * S                     # 36864 samples per core
S_TOT = float(B * NS)          # BN sample count
EPS = 1e-5
N_CORES = 8

GH = 2048                      # samples per dma_gather
NG = NS // GH                  # 18 gathers
NCHUNK = GH // 128             # 16 chunks of 128 samples per gather


def _ap_raw(ap, offset, dims):
    a = ap.copy()
    a.offset = offset
    a.ap = bass_rust.VecI64Pair(dims)
    return a


def build(stage=3, new_idx=True, new_w=True):
    nc = bacc.Bacc("TRN2", target_bir_lowering=False, debug=False,
                   num_devices=N_CORES)
    xpad_d = nc.dram_tensor("xpad", [INC, 66 * 66], F32, kind="ExternalInput")
    x2_d = nc.dram_tensor("x2", [S, 2 * INC], BF16, kind="ExternalInput")
    pwt_d = nc.dram_tensor("pwt", [INC, 9, 2 * N], F32, kind="ExternalInput")
    base4_d = nc.dram_tensor("base4", [128, 32, 2 * N], F32, kind="ExternalInput")
    cwt_d = nc.dram_tensor("cwt", [INC, OUTC], F32, kind="ExternalInput")
    gb_d = nc.dram_tensor("gb", [1, 2 * OUTC], F32, kind="ExternalInput")
    id18_d = nc.dram_tensor("id18", [18, 18], F32, kind="ExternalInput")
    id128_d = nc.dram_tensor("id128", [128, 128], BF16, kind="ExternalInput")
    id128f_d = nc.dram_tensor("id128f", [128, 128], F32, kind="ExternalInput")
    out_d = nc.dram_tensor("out", [OUTC, NS], BF16, kind="ExternalOutput")

    idx_dram = nc.dram_tensor("idx_scratch", [1, NS], F32, kind="Internal")
    w_dram = nc.dram_tensor("w_scratch", [4, NS], BF16, kind="Internal")
    ab_dram = nc.dram_tensor("ab_scratch", [OUTC, 2], F32, kind="Internal")
    y1_dram = nc.dram_tensor("y1_scratch", [128, NS], BF16, kind="Internal")

    with tile.TileContext(nc) as tc:
        with tc.tile_pool(name="persist", bufs=1) as pp, \
             tc.tile_pool(name="work", bufs=1) as wp, \
             tc.tile_pool(name="dram", bufs=1, space="DRAM") as dp:

            # ---------- loads ----------
            pwt = pp.tile([INC, 9, 2 * N], F32)
            nc.sync.dma_start(pwt[:], pwt_d[:])
            base4 = pp.tile([128, 32, 2 * N], F32)
            nc.sync.dma_start(base4[:], base4_d[:])
            cwt = pp.tile([INC, OUTC], F32)
            nc.sync.dma_start(cwt[:], cwt_d[:])
            gb = pp.tile([1, 2 * OUTC], F32)
            nc.sync.dma_start(gb[:], gb_d[:])
            id128 = pp.tile([128, 128], BF16)
            nc.sync.dma_start(id128[:], id128_d[:])
            id128f = pp.tile([128, 128], F32)
            nc.sync.dma_start(id128f[:], id128f_d[:])
            cwt_b = pp.tile([INC, OUTC], BF16)
            nc.vector.tensor_copy(cwt_b[:], cwt[:])
            ones_b = pp.tile([128, 1], BF16)
            nc.vector.memset(ones_b[:], 1.0)
            ones_f = pp.tile([128, 1], F32)
            nc.vector.memset(ones_f[:], 1.0)
            warm_in = dp.tile([1, 4], F32)
            warm_out = dp.tile([1, 4], F32, addr_space="Shared")
            if stage >= 3:
                nc.sync.dma_start(warm_in[:], gb_d[:, 0:4])
                nc.gpsimd.collective_compute(
                    "AllReduce", OP.add,
                    replica_groups=[list(range(N_CORES))],
                    ins=[warm_in[:].opt()],
                    outs=[warm_out[:].opt()])
            posT = pp.tile([128, 32, 2 * N], F32)

            # ---------- p_conv: offsets (18, S) then transpose ----------
            id18 = pp.tile([18, 18], F32)
            nc.sync.dma_start(id18[:], id18_d[:])
            with tc.tile_pool(name="xpadp", bufs=1) as xp, \
                 tc.tile_pool(name="psum_pc", bufs=4, space="PSUM") as pcp:
                xpad = xp.tile([INC, 66 * 66], F32)
                nc.sync.dma_start(xpad[:], xpad_d[:])
                xpad_ap = xpad[:]
                pstride = xpad_ap.ap[0][0]
                base_off = xpad_ap.offset
                offs = xp.tile([18, S], F32)
                for c8 in range(8):
                    acc = pcp.tile([18, 512], F32, tag="pconv")
                    for tap in range(9):
                        dh, dw = tap // 3, tap % 3
                        mov = _ap_raw(xpad_ap,
                                      base_off + (c8 * 8 + dh) * 66 + dw,
                                      [(pstride, 128), (66, 8), (1, 64)])
                        nc.tensor.matmul(acc[:], pwt[:, tap, :], mov,
                                         start=(tap == 0), stop=(tap == 8))
                    nc.vector.tensor_copy(offs[:, c8 * 512:(c8 + 1) * 512],
                                          acc[:])
                for t in range(32):
                    tp = pcp.tile([128, 18], F32, tag="tpose")
                    nc.tensor.transpose(tp[:], offs[:, t * 128:(t + 1) * 128],
                                        id18[:])
                    nc.vector.tensor_copy(posT[:, t, :], tp[:])

            # ---------- positions / weights / indices ----------
            def ts(out, in_, s1, o1, s2=None, o2=None):
                if s2 is None:
                    nc.vector.tensor_scalar(out, in_, s1, None, op0=o1)
                else:
                    nc.vector.tensor_scalar(out, in_, s1, s2, op0=o1, op1=o2)

            _slab_ctr = [0]

            def slab():
                _slab_ctr[0] += 1
                return wpp.tile([128, 32, 2 * N], F32,
                                name=f"slab{_slab_ctr[0]}",
                                tag=f"slab{_slab_ctr[0]}")

            _wpp_cm = tc.tile_pool(name="wrapp", bufs=1)
            wpp = _wpp_cm.__enter__()
            p4 = base4                       # reuse base4 storage for p4
            nc.vector.tensor_add(p4[:], posT[:], base4[:])
            pc4 = slab()
            ts(pc4[:], p4[:], 4.0, OP.max, 67.0, OP.min)
            i32 = wpp.tile([128, 32, 2 * N], mybir.dt.int32)
            nc.vector.tensor_copy(i32[:], p4[:])
            mfr = slab()
            nc.vector.tensor_copy(mfr[:], i32[:])
            f4 = slab()
            nc.vector.tensor_tensor(f4[:], p4[:], mfr[:], op=OP.is_lt)
            nc.vector.tensor_tensor(f4[:], mfr[:], f4[:], op=OP.subtract)
            qlt = mfr                        # reuse
            ts(qlt[:], f4[:], 4.0, OP.max, 67.0, OP.min)
            qrb = slab()
            ts(qrb[:], f4[:], 1.0, OP.add, 4.0, OP.max)
            ts(qrb[:], qrb[:], 67.0, OP.min)
            g04 = slab()
            ts(g04[:], f4[:], 4.0, OP.max, 66.0, OP.min)
            ax = slab()
            nc.vector.tensor_tensor(ax[:], qlt[:], pc4[:], op=OP.subtract)
            ts(ax[:], ax[:], 1.0, OP.add)
            bx = qlt                         # reuse (qlt dead)
            nc.vector.tensor_tensor(bx[:], pc4[:], qrb[:], op=OP.subtract)
            ts(bx[:], bx[:], 1.0, OP.add)
            c1 = pc4                         # reuse (pc4 dead)
            ts(c1[:], f4[:], 66.0, OP.is_le)
            c2 = qrb                         # reuse (qrb dead)
            ts(c2[:], f4[:], 3.0, OP.is_le)
            w0 = f4                          # reuse (f4 dead)
            nc.vector.tensor_tensor(w0[:], ax[:], c1[:], op=OP.mult)
            t0 = slab()
            nc.vector.tensor_tensor(t0[:], bx[:], c2[:], op=OP.mult)
            nc.vector.tensor_add(w0[:], w0[:], t0[:])
            w1 = t0                          # reuse
            nc.vector.tensor_add(w1[:], ax[:], bx[:])
            nc.vector.tensor_tensor(w1[:], w1[:], w0[:], op=OP.subtract)

            # corner weights -> wslab[p, t, j, n] bf16; j: 0=x2 row order
            # (lt, lb, rt, rb) = (w0x*w0y, w1x*w0y, w0x*w1y, w1x*w1y)
            wslab = wpp.tile([128, 32, 4, 9], BF16)
            for j, (wx, wy) in enumerate([(w0, w0), (w1, w0), (w0, w1), (w1, w1)]):
                nc.vector.tensor_tensor(wslab[:, :, j, :], wx[:, :, 0:N],
                                        wy[:, :, N:2 * N], op=OP.mult)

            # idx = g04x*64 + g04y - 260 (4-space shift removal)
            idxf = ax                        # reuse (ax dead), use x-half
            ts(idxf[:, :, 0:N], g04[:, :, 0:N], 64.0, OP.mult, 260.0,
               OP.subtract)
            nc.vector.tensor_add(idxf[:, :, 0:N], idxf[:, :, 0:N],
                                 g04[:, :, N:2 * N])
            # route to k-linear DRAM order, k = 9*(128t + p) + n
            nc.sync.dma_start(
                _ap_raw(idx_dram[:], 0, [(9, 128), (1152, 32), (1, 9)]),
                idxf[:, :, 0:N])
            for j in range(4):
                nc.sync.dma_start(
                    _ap_raw(w_dram[:], j * NS,
                            [(9, 128), (1152, 32), (1, 9)]),
                    wslab[:, :, j, :])
            _wpp_cm.__exit__(None, None, None)

            # gather source AP over x2: overlapping 2-row windows
            x2_src = _ap_raw(x2_d[:], 0, [(2 * INC, S - 1), (1, 4 * INC)])

            # ---------- main loop: gather / combine / PE / y ----------
            y_raw0 = pp.tile([128, NS], BF16)
            chunk_ctr = [0]
            TOT_CHUNKS = NG * NCHUNK
            with tc.tile_pool(name="gather", bufs=2) as gp, \
                 tc.tile_pool(name="xo", bufs=2) as xop, \
                 tc.tile_pool(name="xcmp", bufs=3) as xcp, \
                 tc.tile_pool(name="psum_g", bufs=1, space="PSUM") as gpp, \
                 tc.tile_pool(name="psum_tp", bufs=2, space="PSUM") as tpp, \
                 tc.tile_pool(name="psum_y", bufs=2, space="PSUM") as typ:
                gm1 = gpp.tile([128, 129], F32, tag="gram")
                gpsum = gm1[:, 0:128]
                m1psum = gm1[:, 128:129]
                for g in range(NG):
                    # idx: contiguous [128,16] read, PE transpose to [16,128]
                    idxw = gp.tile([128, GH // 16], I16, tag="idxw")
                    if new_idx:
                        idr = gp.tile([128, 16], F32, tag="idr")
                        nc.sync.dma_start(
                            idr[:],
                            _ap_raw(idx_dram[:], g * GH, [(16, 128), (1, 16)]))
                        idxT = tpp.tile([16, 128], F32, tag="idxT", bufs=1)
                        nc.tensor.transpose(idxT[:], idr[:], id128f[:])
                        ix16 = gp.tile([16, 128], I16, tag="ix16")
                        nc.vector.tensor_copy(ix16[:], idxT[:])
                        for v in range(8):
                            nc.sync.dma_start(idxw[16 * v:16 * (v + 1)],
                                              ix16[:])
                    else:
                        idrf = gp.tile([128, GH // 16], F32, tag="idrf")
                        idx_src = _ap_raw(idx_dram[:], g * GH,
                                          [(1, 16), (16, GH // 16)])
                        for v in range(8):
                            nc.sync.dma_start(idrf[16 * v:16 * (v + 1)],
                                              idx_src)
                        nc.vector.tensor_copy(idxw[:], idrf[:])
                    # weights: per-corner contiguous [16,128] read + transpose
                    w4 = gp.tile([128, 4, NCHUNK], BF16, tag="w4")
                    if new_w:
                        wps = tpp.tile([128, 4, NCHUNK], BF16, tag="wps",
                                       bufs=1)
                        for j in range(4):
                            wrd = gp.tile([16, 128], BF16, tag=f"wrd{j}")
                            nc.sync.dma_start(
                                wrd[:],
                                _ap_raw(w_dram[:], j * NS + g * GH,
                                        [(128, 16), (1, 128)]))
                            nc.tensor.transpose(wps[:, j, :], wrd[:],
                                                id128[0:16, 0:16])
                        nc.scalar.activation(w4[:], wps[:], AF.Copy)
                    else:
                        for j in range(4):
                            nc.sync.dma_start(
                                w4[:, j, :],
                                _ap_raw(w_dram[:], j * NS + g * GH,
                                        [(1, 128), (128, NCHUNK)]))
                    dst = gp.tile([128, NCHUNK, 512], BF16, tag="gdst")
                    nc.gpsimd.dma_gather(
                        dst[:], x2_src, idxw[:], GH, GH, 4 * INC,
                        elem_step=2 * INC, transpose=False,
                        single_packet=False)

                    # bilinear combine on DVE (stride-0 broadcast weights)
                    dd = dst[:]
                    doff, dstr = dd.offset, dd.ap[0][0]
                    ww = w4[:]
                    woff, wstr = ww.offset, ww.ap[0][0]

                    def ds(j):
                        return _ap_raw(dd, doff + j * 128,
                                       [(dstr, 128), (512, NCHUNK), (1, 128)])

                    def wb(j):
                        return _ap_raw(ww, woff + j * NCHUNK,
                                       [(wstr, 128), (1, NCHUNK), (0, 128)])

                    xo = xop.tile([128, NCHUNK, 128], BF16, tag="xo")
                    nc.vector.tensor_tensor(ds(0), ds(0), wb(0), op=OP.mult)
                    nc.vector.tensor_tensor(ds(1), ds(1), wb(1), op=OP.mult)
                    nc.vector.tensor_tensor(ds(0), ds(0), ds(1), op=OP.add)
                    nc.vector.tensor_tensor(ds(2), ds(2), wb(2), op=OP.mult)
                    nc.vector.tensor_tensor(ds(3), ds(3), wb(3), op=OP.mult)
                    nc.vector.tensor_tensor(ds(2), ds(2), ds(3), op=OP.add)
                    nc.vector.tensor_tensor(xo[:], ds(0), ds(2), op=OP.add)

                    # per 4-chunk group: transpose+gram+m1, then y matmuls
                    for grp in range(NCHUNK // 4):
                        xcm = xcp.tile([128, 4, 128], BF16, tag="xcm")
                        for q in range(4):
                            i = grp * 4 + q
                            ci = chunk_ctr[0]
                            chunk_ctr[0] += 1
                            tp = tpp.tile([128, 128], BF16, tag="tp")
                            nc.tensor.transpose(tp[:], xo[:, i, :], id128[:])
                            nc.tensor.matmul(gpsum, xo[:, i, :],
                                             xo[:, i, :],
                                             start=(ci == 0),
                                             stop=(ci == TOT_CHUNKS - 1),
                                             skip_group_check=True)
                            nc.tensor.matmul(m1psum, xo[:, i, :], ones_b[:],
                                             start=(ci == 0),
                                             stop=(ci == TOT_CHUNKS - 1),
                                             skip_group_check=True)
                            nc.scalar.activation(xcm[:, q, :], tp[:], AF.Copy)
                        xmov = xcm[:].rearrange("p a b -> p (a b)")
                        col = g * GH + grp * 512
                        y0p = typ.tile([128, 512], F32, tag="y0")
                        nc.tensor.matmul(y0p[:], cwt_b[:, 0:128], xmov,
                                         start=True, stop=True)
                        y1p = typ.tile([128, 512], F32, tag="y1", bufs=1)
                        nc.tensor.matmul(y1p[:], cwt_b[:, 128:256], xmov,
                                         start=True, stop=True)
                        nc.scalar.activation(y_raw0[:, col:col + 512], y0p[:],
                                             AF.Copy)
                        ystg = xcp.tile([128, 512], BF16, tag="ystg")
                        nc.vector.tensor_copy(ystg[:], y1p[:])
                        nc.sync.dma_start(y1_dram[:, col:col + 512], ystg[:])

                g_sb = wp.tile([128, 129], F32)
                nc.vector.tensor_copy(g_sb[:], gm1[:])

            # ---------- allreduce + BN coefficients ----------
            gsum = wp.tile([128, 129], F32)
            if stage >= 3:
                bounce_in = dp.tile([128, 129], F32)
                bounce_out = dp.tile([128, 129], F32, addr_space="Shared")
                nc.sync.dma_start(bounce_in[:], g_sb[:])
                nc.gpsimd.collective_compute(
                    "AllReduce", OP.add,
                    replica_groups=[list(range(N_CORES))],
                    ins=[bounce_in[:].opt()],
                    outs=[bounce_out[:].opt()])
                nc.sync.dma_start(gsum[:], bounce_out[:])
            else:
                nc.vector.tensor_scalar(gsum[:], g_sb[:], 8.0, None,
                                        op0=OP.mult)

            with tc.tile_pool(name="psum_s", bufs=1, space="PSUM") as sp:
                t1p = sp.tile([128, OUTC], F32)
                nc.tensor.matmul(t1p[:], gsum[:, 0:128], cwt[:],
                                 start=True, stop=True)
                m2 = wp.tile([128, OUTC], F32)
                nc.vector.tensor_tensor(m2[:], cwt[:], t1p[:], op=OP.mult)
                dvp = sp.tile([1, OUTC], F32)
                nc.tensor.matmul(dvp[:], ones_f[:], m2[:], start=True,
                                 stop=True)
                m1yp = sp.tile([1, OUTC], F32)
                nc.tensor.matmul(m1yp[:], gsum[:, 128:129], cwt[:],
                                 start=True, stop=True)

                meanv = wp.tile([1, OUTC], F32)
                ts(meanv[:], m1yp[:], 1.0 / S_TOT, OP.mult)
                varv = wp.tile([1, OUTC], F32)
                ts(varv[:], dvp[:], 1.0 / S_TOT, OP.mult)
                msq = wp.tile([1, OUTC], F32)
                nc.vector.tensor_tensor(msq[:], meanv[:], meanv[:], op=OP.mult)
                nc.vector.tensor_tensor(varv[:], varv[:], msq[:],
                                        op=OP.subtract)
                ts(varv[:], varv[:], EPS, OP.add)
                sd = wp.tile([1, OUTC], F32)
                nc.scalar.activation(sd[:], varv[:], AF.Sqrt)
                rsd = wp.tile([1, OUTC], F32)
                nc.vector.reciprocal(rsd[:], sd[:])
                a_v = wp.tile([1, OUTC], F32)
                nc.vector.tensor_tensor(a_v[:], rsd[:], gb[:, 0:OUTC],
                                        op=OP.mult)
                b_v = wp.tile([1, OUTC], F32)
                nc.vector.tensor_tensor(b_v[:], meanv[:], a_v[:], op=OP.mult)
                nc.vector.tensor_tensor(b_v[:], gb[:, OUTC:2 * OUTC], b_v[:],
                                        op=OP.subtract)

            nc.sync.dma_start(_ap_raw(ab_dram[:], 0, [(2, OUTC)]), a_v[:])
            nc.sync.dma_start(_ap_raw(ab_dram[:], 1, [(2, OUTC)]), b_v[:])
            ab = pp.tile([128, 2, 2], F32)
            nc.sync.dma_start(
                ab[:], _ap_raw(ab_dram[:], 0, [(2, 128), (256, 2), (1, 2)]))

            # ---------- silu epilogue ----------
            YB = 4096
            with tc.tile_pool(name="ybuf", bufs=3) as yb, \
                 tc.tile_pool(name="y1rd", bufs=3) as yr:
                y1t = []
                for blk in range(NS // YB):
                    y1blk = yr.tile([128, YB], BF16, tag="y1blk")
                    nc.sync.dma_start(y1blk[:],
                                      y1_dram[:, blk * YB:(blk + 1) * YB])
                    y1t.append(y1blk)
                    ybuf = yb.tile([128, YB], BF16, tag="yb")
                    nc.scalar.activation(
                        ybuf[:], y_raw0[:, blk * YB:(blk + 1) * YB],
                        AF.Silu, scale=ab[:, 0, 0:1], bias=ab[:, 0, 1:2])
                    out_ap = _ap_raw(
                        out_d[:], blk * YB,
                        [(NS, 128), (1, YB)])
                    nc.sync.dma_start(out_ap, ybuf[:])
                    ybuf1 = yb.tile([128, YB], BF16, tag="yb")
                    nc.scalar.activation(
                        ybuf1[:], y1blk[:],
                        AF.Silu, scale=ab[:, 1, 0:1], bias=ab[:, 1, 1:2])
                    out_ap1 = _ap_raw(
                        out_d[:], 128 * NS + blk * YB,
                        [(NS, 128), (1, YB)])
                    nc.sync.dma_start(out_ap1, ybuf1[:])

    nc.compile()
    return nc


def prep_inputs(x, pw, pb, cw, gamma, beta):
    x = np.asarray(x, np.float32)
    pw = np.asarray(pw, np.float32)
    pb = np.asarray(pb, np.float32)
    cw = np.asarray(cw, np.float32)
    gamma = np.asarray(gamma, np.float32)
    beta = np.asarray(beta, np.float32)

    pwt = np.ascontiguousarray(
        pw.reshape(2 * N, INC, 9).transpose(1, 2, 0))      # (128, 9, 18)

    angles = np.linspace(0.0, 2.0 * math.pi, N + 1, dtype=np.float64)[:-1]
    pn = np.concatenate([np.cos(angles), np.sin(angles)]).astype(np.float32)
    p_idx = np.arange(128)
    t_idx = np.arange(32)
    hh = (2 * t_idx[None, :] + (p_idx[:, None] >= 64)).astype(np.float32)
    ww = np.broadcast_to((p_idx % 64).astype(np.float32)[:, None], (128, 32))
    base4 = np.zeros((128, 32, 2 * N), np.float32)
    base4[:, :, 0:N] = hh[:, :, None] + (pn[0:N] + pb[0:N])[None, None, :] + 4.0
    base4[:, :, N:] = ww[:, :, None] + (pn[N:] + pb[N:])[None, None, :] + 4.0

    cwt = np.ascontiguousarray(cw[:, :, 0, 0].T)           # (128, 256)
    gb = np.concatenate([gamma, beta])[None, :]            # (1, 512)
    id128 = np.eye(128, dtype=np.float32).astype(ml_dtypes.bfloat16)

    in_maps = []
    for b in range(B):
        xb = x[b].reshape(INC, S)
        xpad = np.zeros((INC, 66, 66), np.float32)
        xpad[:, 1:65, 1:65] = x[b]
        xT = np.ascontiguousarray(xb.T).astype(ml_dtypes.bfloat16)  # (4096, 128)
        x2 = np.zeros((S, 2 * INC), ml_dtypes.bfloat16)
        x2[:, 0:INC] = xT
        x2[:S - 64, INC:] = xT[64:]
        in_maps.append(dict(
            xpad=np.ascontiguousarray(xpad.reshape(INC, 66 * 66)), x2=x2,
            pwt=pwt, base4=base4, cwt=cwt, gb=gb,
            id18=np.eye(18, dtype=np.float32), id128=id128,
            id128f=np.eye(128, dtype=np.float32)))
    return in_maps


_NC_CACHE = {}


def kernel(x, pw, pb, cw, gamma, beta):
    import os
    if "nc" not in _NC_CACHE:
        _NC_CACHE["nc"] = build(
            new_idx=os.environ.get("NEWIDX", "1") == "1",
            new_w=os.environ.get("NEWW", "1") == "1")
    nc = _NC_CACHE["nc"]
    in_maps = prep_inputs(x, pw, pb, cw, gamma, beta)
    res = run_bass_kernel_spmd(nc, in_maps, core_ids=list(range(N_CORES)))
    out = np.stack([
        np.asarray(res.results[b]["out"]).astype(np.float32).reshape(
            OUTC, H, W * N)
        for b in range(B)])
    return out


# revision 14
# speedup vs baseline: 1.2919x; 1.0879x over previous
"""AKConv TRN2 kernel: 8-core data-parallel over batch.

Sample-major gather architecture: dma_gather(transpose=False) puts each
sample (4 corners x 128ch) on one partition; bilinear combine on DVE via
stride-0 broadcast weights; PE transposes chunks back to channel-major,
accumulates the BN gram + row sums off the same stationary, and runs the
1x1 conv for both outc halves inline. Tail: allreduce -> BN coeffs ->
SiLU -> output DMA.
"""
import sys
sys.path.insert(0, "/opt/trn_rl_repo")
import math
import numpy as np
import ml_dtypes
import bass_rust
import concourse.bass as bass
import concourse.tile as tile
from concourse import bacc, mybir
from concourse.bass_utils import run_bass_kernel_spmd

F32 = mybir.dt.float32
BF16 = mybir.dt.bfloat16
I16 = mybir.dt.int16
AF = mybir.ActivationFunctionType
OP = mybir.AluOpType

B, INC, H, W = 8, 128, 64, 64
OUTC, N = 256, 9
S = H * W                      # 4096 pixels per core
NS = N * S                     # 36864 samples per core
S_TOT = float(B * NS)          # BN sample count
EPS = 1e-5
N_CORES = 8

GH = 2048                      # samples per dma_gather
NG = NS // GH                  # 18 gathers
NCHUNK = GH // 128             # 16 chunks of 128 samples per gather


def _ap_raw(ap, offset, dims):
    a = ap.copy()
    a.offset = offset
    a.ap = bass_rust.VecI64Pair(dims)
    return a


def build(stage=3, new_idx=True, new_w=True):
    nc = bacc.Bacc("TRN2", target_bir_lowering=False, debug=False,
                   num_devices=N_CORES)
    xpad_d = nc.dram_tensor("xpad", [INC, 66 * 66], F32, kind="ExternalInput")
    x2_d = nc.dram_tensor("x2", [S, 2 * INC], BF16, kind="ExternalInput")
    pwt_d = nc.dram_tensor("pwt", [INC, 9, 2 * N], F32, kind="ExternalInput")
    base4_d = nc.dram_tensor("base4", [128, 32, 2 * N], F32, kind="ExternalInput")
    cwt_d = nc.dram_tensor("cwt", [INC, OUTC], F32, kind="ExternalInput")
    gb_d = nc.dram_tensor("gb", [1, 2 * OUTC], F32, kind="ExternalInput")
    id18_d = nc.dram_tensor("id18", [18, 18], F32, kind="ExternalInput")
    id128_d = nc.dram_tensor("id128", [128, 128], BF16, kind="ExternalInput")
    id128f_d = nc.dram_tensor("id128f", [128, 128], F32, kind="ExternalInput")
    out_d = nc.dram_tensor("out", [OUTC, NS], BF16, kind="ExternalOutput")

    idx_dram = nc.dram_tensor("idx_scratch", [1, NS], F32, kind="Internal")
    w_dram = nc.dram_tensor("w_scratch", [4, NS], BF16, kind="Internal")
    ab_dram = nc.dram_tensor("ab_scratch", [OUTC, 2], F32, kind="Internal")
    y1_dram = nc.dram_tensor("y1_scratch", [128, NS], BF16, kind="Internal")

    with tile.TileContext(nc) as tc:
        with tc.tile_pool(name="persist", bufs=1) as pp, \
             tc.tile_pool(name="work", bufs=1) as wp, \
             tc.tile_pool(name="dram", bufs=1, space="DRAM") as dp:

            # ---------- loads ----------
            pwt = pp.tile([INC, 9, 2 * N], F32)
            nc.sync.dma_start(pwt[:], pwt_d[:])
            base4 = pp.tile([128, 32, 2 * N], F32)
            nc.sync.dma_start(base4[:], base4_d[:])
            cwt = pp.tile([INC, OUTC], F32)
            nc.sync.dma_start(cwt[:], cwt_d[:])
            gb = pp.tile([1, 2 * OUTC], F32)
            nc.sync.dma_start(gb[:], gb_d[:])
            id128 = pp.tile([128, 128], BF16)
            nc.sync.dma_start(id128[:], id128_d[:])
            id128f = pp.tile([128, 128], F32)
            nc.sync.dma_start(id128f[:], id128f_d[:])
            cwt_b = pp.tile([INC, OUTC], BF16)
            nc.vector.tensor_copy(cwt_b[:], cwt[:])
            ones_b = pp.tile([128, 1], BF16)
            nc.vector.memset(ones_b[:], 1.0)
            ones_f = pp.tile([128, 1], F32)
            nc.vector.memset(ones_f[:], 1.0)
            warm_in = dp.tile([1, 4], F32)
            warm_out = dp.tile([1, 4], F32, addr_space="Shared")
            if stage >= 3:
                nc.sync.dma_start(warm_in[:], gb_d[:, 0:4])
                nc.gpsimd.collective_compute(
                    "AllReduce", OP.add,
                    replica_groups=[list(range(N_CORES))],
                    ins=[warm_in[:].opt()],
                    outs=[warm_out[:].opt()])
            posT = pp.tile([128, 32, 2 * N], F32)

            # ---------- p_conv: offsets (18, S) then transpose ----------
            id18 = pp.tile([18, 18], F32)
            nc.sync.dma_start(id18[:], id18_d[:])
            with tc.tile_pool(name="xpadp", bufs=1) as xp, \
                 tc.tile_pool(name="psum_pc", bufs=4, space="PSUM") as pcp:
                xpad = xp.tile([INC, 66 * 66], F32)
                nc.sync.dma_start(xpad[:], xpad_d[:])
                xpad_ap = xpad[:]
                pstride = xpad_ap.ap[0][0]
                base_off = xpad_ap.offset
                offs = xp.tile([18, S], F32)
                for c8 in range(8):
                    acc = pcp.tile([18, 512], F32, tag="pconv")
                    for tap in range(9):
                        dh, dw = tap // 3, tap % 3
                        mov = _ap_raw(xpad_ap,
                                      base_off + (c8 * 8 + dh) * 66 + dw,
                                      [(pstride, 128), (66, 8), (1, 64)])
                        nc.tensor.matmul(acc[:], pwt[:, tap, :], mov,
                                         start=(tap == 0), stop=(tap == 8))
                    nc.vector.tensor_copy(offs[:, c8 * 512:(c8 + 1) * 512],
                                          acc[:])
                for t in range(32):
                    tp = pcp.tile([128, 18], F32, tag="tpose")
                    nc.tensor.transpose(tp[:], offs[:, t * 128:(t + 1) * 128],
                                        id18[:])
                    nc.vector.tensor_copy(posT[:, t, :], tp[:])

            # ---------- positions / weights / indices ----------
            def ts(out, in_, s1, o1, s2=None, o2=None):
                if s2 is None:
                    nc.vector.tensor_scalar(out, in_, s1, None, op0=o1)
                else:
                    nc.vector.tensor_scalar(out, in_, s1, s2, op0=o1, op1=o2)

            _slab_ctr = [0]

            def slab():
                _slab_ctr[0] += 1
                return wpp.tile([128, 32, 2 * N], F32,
                                name=f"slab{_slab_ctr[0]}",
                                tag=f"slab{_slab_ctr[0]}")

            _wpp_cm = tc.tile_pool(name="wrapp", bufs=1)
            wpp = _wpp_cm.__enter__()
            p4 = base4                       # reuse base4 storage for p4
            nc.vector.tensor_add(p4[:], posT[:], base4[:])
            pc4 = slab()
            ts(pc4[:], p4[:], 4.0, OP.max, 67.0, OP.min)
            i32 = wpp.tile([128, 32, 2 * N], mybir.dt.int32)
            nc.vector.tensor_copy(i32[:], p4[:])
            mfr = slab()
            nc.vector.tensor_copy(mfr[:], i32[:])
            f4 = slab()
            nc.vector.tensor_tensor(f4[:], p4[:], mfr[:], op=OP.is_lt)
            nc.vector.tensor_tensor(f4[:], mfr[:], f4[:], op=OP.subtract)
            qlt = mfr                        # reuse
            ts(qlt[:], f4[:], 4.0, OP.max, 67.0, OP.min)
            qrb = slab()
            ts(qrb[:], f4[:], 1.0, OP.add, 4.0, OP.max)
            ts(qrb[:], qrb[:], 67.0, OP.min)
            g04 = slab()
            ts(g04[:], f4[:], 4.0, OP.max, 66.0, OP.min)
            ax = slab()
            nc.vector.tensor_tensor(ax[:], qlt[:], pc4[:], op=OP.subtract)
            ts(ax[:], ax[:], 1.0, OP.add)
            bx = qlt                         # reuse (qlt dead)
            nc.vector.tensor_tensor(bx[:], pc4[:], qrb[:], op=OP.subtract)
            ts(bx[:], bx[:], 1.0, OP.add)
            c1 = pc4                         # reuse (pc4 dead)
            ts(c1[:], f4[:], 66.0, OP.is_le)
            c2 = qrb                         # reuse (qrb dead)
            ts(c2[:], f4[:], 3.0, OP.is_le)
            w0 = f4                          # reuse (f4 dead)
            nc.vector.tensor_tensor(w0[:], ax[:], c1[:], op=OP.mult)
            t0 = slab()
            nc.vector.tensor_tensor(t0[:], bx[:], c2[:], op=OP.mult)
            nc.vector.tensor_add(w0[:], w0[:], t0[:])
            w1 = t0                          # reuse
            nc.vector.tensor_add(w1[:], ax[:], bx[:])
            nc.vector.tensor_tensor(w1[:], w1[:], w0[:], op=OP.subtract)

            # corner weights -> wslab[p, t, j, n] bf16; j: 0=x2 row order
            # (lt, lb, rt, rb) = (w0x*w0y, w1x*w0y, w0x*w1y, w1x*w1y)
            wslab = wpp.tile([128, 32, 4, 9], BF16)
            for j, (wx, wy) in enumerate([(w0, w0), (w1, w0), (w0, w1), (w1, w1)]):
                nc.vector.tensor_tensor(wslab[:, :, j, :], wx[:, :, 0:N],
                                        wy[:, :, N:2 * N], op=OP.mult)

            # idx = g04x*64 + g04y - 260 (4-space shift removal)
            idxf = ax                        # reuse (ax dead), use x-half
            ts(idxf[:, :, 0:N], g04[:, :, 0:N], 64.0, OP.mult, 260.0,
               OP.subtract)
            nc.vector.tensor_add(idxf[:, :, 0:N], idxf[:, :, 0:N],
                                 g04[:, :, N:2 * N])
            # route to k-linear DRAM order, k = 9*(128t + p) + n
            nc.sync.dma_start(
                _ap_raw(idx_dram[:], 0, [(9, 128), (1152, 32), (1, 9)]),
                idxf[:, :, 0:N])
            for j in range(4):
                nc.sync.dma_start(
                    _ap_raw(w_dram[:], j * NS,
                            [(9, 128), (1152, 32), (1, 9)]),
                    wslab[:, :, j, :])
            _wpp_cm.__exit__(None, None, None)

            # gather source AP over x2: overlapping 2-row windows
            x2_src = _ap_raw(x2_d[:], 0, [(2 * INC, S - 1), (1, 4 * INC)])

            # ---------- main loop: gather / combine / PE / y ----------
            y_raw0 = pp.tile([128, NS], BF16)
            chunk_ctr = [0]
            TOT_CHUNKS = NG * NCHUNK
            with tc.tile_pool(name="gather", bufs=2) as gp, \
                 tc.tile_pool(name="xo", bufs=2) as xop, \
                 tc.tile_pool(name="xcmp", bufs=3) as xcp, \
                 tc.tile_pool(name="psum_g", bufs=1, space="PSUM") as gpp, \
                 tc.tile_pool(name="psum_tp", bufs=2, space="PSUM") as tpp, \
                 tc.tile_pool(name="psum_y", bufs=2, space="PSUM") as typ:
                gm1 = gpp.tile([128, 129], F32, tag="gram")
                gpsum = gm1[:, 0:128]
                m1psum = gm1[:, 128:129]
                for gpr in range(NG // 2):
                  # weights for a PAIR of gathers: one K=128 PE transpose
                  if True:
                    w4pair = gp.tile([128, 128], BF16, tag="w4pair")
                    if new_w:
                        wrd2 = gp.tile([128, 128], BF16, tag="wrd2")
                        for g2 in range(2):
                            for j in range(4):
                                nc.sync.dma_start(
                                    wrd2[g2 * 64 + j * 16:
                                         g2 * 64 + j * 16 + 16, :],
                                    _ap_raw(w_dram[:],
                                            j * NS + (2 * gpr + g2) * GH,
                                            [(128, 16), (1, 128)]))
                        wps2 = tpp.tile([128, 128], BF16, tag="wps2", bufs=1)
                        nc.tensor.transpose(wps2[:], wrd2[:], id128[:])
                        nc.scalar.activation(w4pair[:], wps2[:], AF.Copy)
                    else:
                        for g2 in range(2):
                            for j in range(4):
                                nc.sync.dma_start(
                                    _ap_raw(w4pair[:],
                                            w4pair[:].offset + g2 * 64 + j * 16,
                                            [(w4pair[:].ap[0][0], 128),
                                             (1, 16)]),
                                    _ap_raw(w_dram[:],
                                            j * NS + (2 * gpr + g2) * GH,
                                            [(1, 128), (128, NCHUNK)]))
                  for g2 in range(2):
                    g = 2 * gpr + g2
                    # idx: contiguous [128,16] read, PE transpose to [16,128]
                    idxw = gp.tile([128, GH // 16], I16, tag="idxw")
                    if new_idx:
                        idr = gp.tile([128, 16], F32, tag="idr")
                        nc.sync.dma_start(
                            idr[:],
                            _ap_raw(idx_dram[:], g * GH, [(16, 128), (1, 16)]))
                        idxT = tpp.tile([16, 128], F32, tag="idxT", bufs=1)
                        nc.tensor.transpose(idxT[:], idr[:], id128f[:])
                        ix16 = gp.tile([16, 128], I16, tag="ix16")
                        nc.vector.tensor_copy(ix16[:], idxT[:])
                        for v in range(8):
                            nc.sync.dma_start(idxw[16 * v:16 * (v + 1)],
                                              ix16[:])
                    else:
                        idrf = gp.tile([128, GH // 16], F32, tag="idrf")
                        idx_src = _ap_raw(idx_dram[:], g * GH,
                                          [(1, 16), (16, GH // 16)])
                        for v in range(8):
                            nc.sync.dma_start(idrf[16 * v:16 * (v + 1)],
                                              idx_src)
                        nc.vector.tensor_copy(idxw[:], idrf[:])
                    dst = gp.tile([128, NCHUNK, 512], BF16, tag="gdst")
                    nc.gpsimd.dma_gather(
                        dst[:], x2_src, idxw[:], GH, GH, 4 * INC,
                        elem_step=2 * INC, transpose=False,
                        single_packet=False)

                    # bilinear combine on DVE (stride-0 broadcast weights)
                    dd = dst[:]
                    doff, dstr = dd.offset, dd.ap[0][0]
                    ww = w4pair[:]
                    woff, wstr = ww.offset + g2 * 64, ww.ap[0][0]

                    def ds(j):
                        return _ap_raw(dd, doff + j * 128,
                                       [(dstr, 128), (512, NCHUNK), (1, 128)])

                    def wb(j):
                        return _ap_raw(ww, woff + j * 16,
                                       [(wstr, 128), (1, 16), (0, 128)])

                    xo = xop.tile([128, NCHUNK, 128], BF16, tag="xo")
                    nc.vector.tensor_tensor(ds(0), ds(0), wb(0), op=OP.mult)
                    nc.vector.tensor_tensor(ds(1), ds(1), wb(1), op=OP.mult)
                    nc.vector.tensor_tensor(ds(0), ds(0), ds(1), op=OP.add)
                    nc.vector.tensor_tensor(ds(2), ds(2), wb(2), op=OP.mult)
                    nc.vector.tensor_tensor(ds(3), ds(3), wb(3), op=OP.mult)
                    nc.vector.tensor_tensor(ds(2), ds(2), ds(3), op=OP.add)
                    nc.vector.tensor_tensor(xo[:], ds(0), ds(2), op=OP.add)

                    # per 4-chunk group: transpose+gram+m1, then y matmuls
                    for grp in range(NCHUNK // 4):
                        xcm = xcp.tile([128, 4, 128], BF16, tag="xcm")
                        for q in range(4):
                            i = grp * 4 + q
                            ci = chunk_ctr[0]
                            chunk_ctr[0] += 1
                            tp = tpp.tile([128, 128], BF16, tag="tp")
                            nc.tensor.transpose(tp[:], xo[:, i, :], id128[:])
                            nc.tensor.matmul(gpsum, xo[:, i, :],
                                             xo[:, i, :],
                                             start=(ci == 0),
                                             stop=(ci == TOT_CHUNKS - 1),
                                             skip_group_check=True)
                            nc.tensor.matmul(m1psum, xo[:, i, :], ones_b[:],
                                             start=(ci == 0),
                                             stop=(ci == TOT_CHUNKS - 1),
                                             skip_group_check=True)
                            nc.scalar.activation(xcm[:, q, :], tp[:], AF.Copy)
                        xmov = xcm[:].rearrange("p a b -> p (a b)")
                        col = g * GH + grp * 512
                        y0p = typ.tile([128, 512], F32, tag="y0")
                        nc.tensor.matmul(y0p[:], cwt_b[:, 0:128], xmov,
                                         start=True, stop=True)
                        y1p = typ.tile([128, 512], F32, tag="y1", bufs=1)
                        nc.tensor.matmul(y1p[:], cwt_b[:, 128:256], xmov,
                                         start=True, stop=True)
                        nc.scalar.activation(y_raw0[:, col:col + 512], y0p[:],
                                             AF.Copy)
                        ystg = xcp.tile([128, 512], BF16, tag="ystg")
                        nc.vector.tensor_copy(ystg[:], y1p[:])
                        nc.sync.dma_start(y1_dram[:, col:col + 512], ystg[:])

                g_sb = wp.tile([128, 129], F32)
                nc.vector.tensor_copy(g_sb[:], gm1[:])

            # ---------- allreduce + BN coefficients ----------
            gsum = wp.tile([128, 129], F32)
            if stage >= 3:
                bounce_in = dp.tile([128, 129], F32)
                bounce_out = dp.tile([128, 129], F32, addr_space="Shared")
                nc.sync.dma_start(bounce_in[:], g_sb[:])
                nc.gpsimd.collective_compute(
                    "AllReduce", OP.add,
                    replica_groups=[list(range(N_CORES))],
                    ins=[bounce_in[:].opt()],
                    outs=[bounce_out[:].opt()])
                nc.sync.dma_start(gsum[:], bounce_out[:])
            else:
                nc.vector.tensor_scalar(gsum[:], g_sb[:], 8.0, None,
                                        op0=OP.mult)

            with tc.tile_pool(name="psum_s", bufs=1, space="PSUM") as sp:
                t1p = sp.tile([128, OUTC], F32)
                nc.tensor.matmul(t1p[:], gsum[:, 0:128], cwt[:],
                                 start=True, stop=True)
                m2 = wp.tile([128, OUTC], F32)
                nc.vector.tensor_tensor(m2[:], cwt[:], t1p[:], op=OP.mult)
                dvp = sp.tile([1, OUTC], F32)
                nc.tensor.matmul(dvp[:], ones_f[:], m2[:], start=True,
                                 stop=True)
                m1yp = sp.tile([1, OUTC], F32)
                nc.tensor.matmul(m1yp[:], gsum[:, 128:129], cwt[:],
                                 start=True, stop=True)

                meanv = wp.tile([1, OUTC], F32)
                ts(meanv[:], m1yp[:], 1.0 / S_TOT, OP.mult)
                varv = wp.tile([1, OUTC], F32)
                ts(varv[:], dvp[:], 1.0 / S_TOT, OP.mult)
                msq = wp.tile([1, OUTC], F32)
                nc.vector.tensor_tensor(msq[:], meanv[:], meanv[:], op=OP.mult)
                nc.vector.tensor_tensor(varv[:], varv[:], msq[:],
                                        op=OP.subtract)
                ts(varv[:], varv[:], EPS, OP.add)
                sd = wp.tile([1, OUTC], F32)
                nc.scalar.activation(sd[:], varv[:], AF.Sqrt)
                rsd = wp.tile([1, OUTC], F32)
                nc.vector.reciprocal(rsd[:], sd[:])
                a_v = wp.tile([1, OUTC], F32)
                nc.vector.tensor_tensor(a_v[:], rsd[:], gb[:, 0:OUTC],
                                        op=OP.mult)
                b_v = wp.tile([1, OUTC], F32)
                nc.vector.tensor_tensor(b_v[:], meanv[:], a_v[:], op=OP.mult)
                nc.vector.tensor_tensor(b_v[:], gb[:, OUTC:2 * OUTC], b_v[:],
                                        op=OP.subtract)

            nc.sync.dma_start(_ap_raw(ab_dram[:], 0, [(2, OUTC)]), a_v[:])
            nc.sync.dma_start(_ap_raw(ab_dram[:], 1, [(2, OUTC)]), b_v[:])
            ab = pp.tile([128, 2, 2], F32)
            nc.sync.dma_start(
                ab[:], _ap_raw(ab_dram[:], 0, [(2, 128), (256, 2), (1, 2)]))

            # ---------- silu epilogue ----------
            YB = 4096
            with tc.tile_pool(name="ybuf", bufs=3) as yb, \
                 tc.tile_pool(name="y1rd", bufs=3) as yr:
                y1t = []
                for blk in range(NS // YB):
                    y1blk = yr.tile([128, YB], BF16, tag="y1blk")
                    nc.sync.dma_start(y1blk[:],
                                      y1_dram[:, blk * YB:(blk + 1) * YB])
                    y1t.append(y1blk)
                    ybuf = yb.tile([128, YB], BF16, tag="yb")
                    nc.scalar.activation(
                        ybuf[:], y_raw0[:, blk * YB:(blk + 1) * YB],
                        AF.Silu, scale=ab[:, 0, 0:1], bias=ab[:, 0, 1:2])
                    out_ap = _ap_raw(
                        out_d[:], blk * YB,
                        [(NS, 128), (1, YB)])
                    nc.sync.dma_start(out_ap, ybuf[:])
                    ybuf1 = yb.tile([128, YB], BF16, tag="yb")
                    nc.scalar.activation(
                        ybuf1[:], y1blk[:],
                        AF.Silu, scale=ab[:, 1, 0:1], bias=ab[:, 1, 1:2])
                    out_ap1 = _ap_raw(
                        out_d[:], 128 * NS + blk * YB,
                        [(NS, 128), (1, YB)])
                    nc.sync.dma_start(out_ap1, ybuf1[:])

    nc.compile()
    return nc


def prep_inputs(x, pw, pb, cw, gamma, beta):
    x = np.asarray(x, np.float32)
    pw = np.asarray(pw, np.float32)
    pb = np.asarray(pb, np.float32)
    cw = np.asarray(cw, np.float32)
    gamma = np.asarray(gamma, np.float32)
    beta = np.asarray(beta, np.float32)

    pwt = np.ascontiguousarray(
        pw.reshape(2 * N, INC, 9).transpose(1, 2, 0))      # (128, 9, 18)

    angles = np.linspace(0.0, 2.0 * math.pi, N + 1, dtype=np.float64)[:-1]
    pn = np.concatenate([np.cos(angles), np.sin(angles)]).astype(np.float32)
    p_idx = np.arange(128)
    t_idx = np.arange(32)
    hh = (2 * t_idx[None, :] + (p_idx[:, None] >= 64)).astype(np.float32)
    ww = np.broadcast_to((p_idx % 64).astype(np.float32)[:, None], (128, 32))
    base4 = np.zeros((128, 32, 2 * N), np.float32)
    base4[:, :, 0:N] = hh[:, :, None] + (pn[0:N] + pb[0:N])[None, None, :] + 4.0
    base4[:, :, N:] = ww[:, :, None] + (pn[N:] + pb[N:])[None, None, :] + 4.0

    cwt = np.ascontiguousarray(cw[:, :, 0, 0].T)           # (128, 256)
    gb = np.concatenate([gamma, beta])[None, :]            # (1, 512)
    id128 = np.eye(128, dtype=np.float32).astype(ml_dtypes.bfloat16)

    in_maps = []
    for b in range(B):
        xb = x[b].reshape(INC, S)
        xpad = np.zeros((INC, 66, 66), np.float32)
        xpad[:, 1:65, 1:65] = x[b]
        xT = np.ascontiguousarray(xb.T).astype(ml_dtypes.bfloat16)  # (4096, 128)
        x2 = np.zeros((S, 2 * INC), ml_dtypes.bfloat16)
        x2[:, 0:INC] = xT
        x2[:S - 64, INC:] = xT[64:]
        in_maps.append(dict(
            xpad=np.ascontiguousarray(xpad.reshape(INC, 66 * 66)), x2=x2,
            pwt=pwt, base4=base4, cwt=cwt, gb=gb,
            id18=np.eye(18, dtype=np.float32), id128=id128,
            id128f=np.eye(128, dtype=np.float32)))
    return in_maps


_NC_CACHE = {}


def kernel(x, pw, pb, cw, gamma, beta):
    import os
    if "nc" not in _NC_CACHE:
        _NC_CACHE["nc"] = build(
            new_idx=os.environ.get("NEWIDX", "1") == "1",
            new_w=os.environ.get("NEWW", "1") == "1")
    nc = _NC_CACHE["nc"]
    in_maps = prep_inputs(x, pw, pb, cw, gamma, beta)
    res = run_bass_kernel_spmd(nc, in_maps, core_ids=list(range(N_CORES)))
    out = np.stack([
        np.asarray(res.results[b]["out"]).astype(np.float32).reshape(
            OUTC, H, W * N)
        for b in range(B)])
    return out


# revision 16
# speedup vs baseline: 1.4956x; 1.1577x over previous
"""AKConv TRN2 kernel: 8-core data-parallel over batch.

Sample-major gather architecture: dma_gather(transpose=False) puts each
sample (4 corners x 128ch) on one partition; bilinear combine on DVE via
stride-0 broadcast weights; PE transposes chunks back to channel-major,
accumulates the BN gram + row sums off the same stationary, and runs the
1x1 conv for both outc halves inline. Tail: allreduce -> BN coeffs ->
SiLU -> output DMA.
"""
import sys
sys.path.insert(0, "/opt/trn_rl_repo")
import math
import numpy as np
import ml_dtypes
import bass_rust
import concourse.bass as bass
import concourse.tile as tile
from concourse import bacc, mybir
from concourse.bass_utils import run_bass_kernel_spmd

F32 = mybir.dt.float32
BF16 = mybir.dt.bfloat16
I16 = mybir.dt.int16
AF = mybir.ActivationFunctionType
OP = mybir.AluOpType

B, INC, H, W = 8, 128, 64, 64
OUTC, N = 256, 9
S = H * W                      # 4096 pixels per core
NS = N * S                     # 36864 samples per core
S_TOT = float(B * NS)          # BN sample count
EPS = 1e-5
N_CORES = 8

GH = 2048                      # samples per dma_gather
NG = NS // GH                  # 18 gathers
NCHUNK = GH // 128             # 16 chunks of 128 samples per gather


def _ap_raw(ap, offset, dims):
    a = ap.copy()
    a.offset = offset
    a.ap = bass_rust.VecI64Pair(dims)
    return a


def build(stage=3, new_idx=True, new_w=True):
    nc = bacc.Bacc("TRN2", target_bir_lowering=False, debug=False,
                   num_devices=N_CORES)
    xpad_d = nc.dram_tensor("xpad", [INC, 66 * 66], BF16, kind="ExternalInput")
    x2_d = nc.dram_tensor("x2", [S, 2 * INC], BF16, kind="ExternalInput")
    pwt_d = nc.dram_tensor("pwt", [INC, 9, 2 * N], F32, kind="ExternalInput")
    base4_d = nc.dram_tensor("base4", [128, 32, 2 * N], F32, kind="ExternalInput")
    cwt_d = nc.dram_tensor("cwt", [INC, OUTC], F32, kind="ExternalInput")
    gb_d = nc.dram_tensor("gb", [1, 2 * OUTC], F32, kind="ExternalInput")
    id18_d = nc.dram_tensor("id18", [18, 18], F32, kind="ExternalInput")
    id128_d = nc.dram_tensor("id128", [128, 128], BF16, kind="ExternalInput")
    id128f_d = nc.dram_tensor("id128f", [128, 128], F32, kind="ExternalInput")
    out_d = nc.dram_tensor("out", [OUTC, NS], BF16, kind="ExternalOutput")

    idx_dram = nc.dram_tensor("idx_scratch", [1, NS], F32, kind="Internal")
    w_dram = nc.dram_tensor("w_scratch", [4, NS], BF16, kind="Internal")
    ab_dram = nc.dram_tensor("ab_scratch", [OUTC, 2], F32, kind="Internal")
    y1_dram = nc.dram_tensor("y1_scratch", [128, NS], BF16, kind="Internal")

    with tile.TileContext(nc) as tc:
        with tc.tile_pool(name="persist", bufs=1) as pp, \
             tc.tile_pool(name="work", bufs=1) as wp, \
             tc.tile_pool(name="dram", bufs=1, space="DRAM") as dp:

            # ---------- loads ----------
            pwt = pp.tile([INC, 9, 2 * N], F32)
            nc.sync.dma_start(pwt[:], pwt_d[:])
            pwt_b = pp.tile([INC, 9, 2 * N], BF16)
            nc.vector.tensor_copy(pwt_b[:], pwt[:])
            base4 = pp.tile([128, 32, 2 * N], F32)
            nc.sync.dma_start(base4[:], base4_d[:])
            cwt = pp.tile([INC, OUTC], F32)
            nc.sync.dma_start(cwt[:], cwt_d[:])
            gb = pp.tile([1, 2 * OUTC], F32)
            nc.sync.dma_start(gb[:], gb_d[:])
            id128 = pp.tile([128, 128], BF16)
            nc.sync.dma_start(id128[:], id128_d[:])
            id128f = pp.tile([128, 128], F32)
            nc.sync.dma_start(id128f[:], id128f_d[:])
            cwt_b = pp.tile([INC, OUTC], BF16)
            nc.vector.tensor_copy(cwt_b[:], cwt[:])
            ones_b = pp.tile([128, 1], BF16)
            nc.vector.memset(ones_b[:], 1.0)
            ones_f = pp.tile([128, 1], F32)
            nc.vector.memset(ones_f[:], 1.0)
            warm_in = dp.tile([1, 4], F32)
            warm_out = dp.tile([1, 4], F32, addr_space="Shared")
            if stage >= 3:
                nc.sync.dma_start(warm_in[:], gb_d[:, 0:4])
                nc.gpsimd.collective_compute(
                    "AllReduce", OP.add,
                    replica_groups=[list(range(N_CORES))],
                    ins=[warm_in[:].opt()],
                    outs=[warm_out[:].opt()])
            posT = pp.tile([128, 32, 2 * N], F32)

            # ---------- p_conv: offsets (18, S) then transpose ----------
            id18 = pp.tile([18, 18], F32)
            nc.sync.dma_start(id18[:], id18_d[:])
            with tc.tile_pool(name="xpadp", bufs=1) as xp, \
                 tc.tile_pool(name="psum_pc", bufs=4, space="PSUM") as pcp:
                xpad = xp.tile([INC, 66 * 66], BF16)
                nc.sync.dma_start(xpad[:], xpad_d[:])
                xpad_ap = xpad[:]
                pstride = xpad_ap.ap[0][0]
                base_off = xpad_ap.offset
                offs = xp.tile([18, S], F32)
                for c8 in range(8):
                    acc = pcp.tile([18, 512], F32, tag="pconv")
                    for tap in range(9):
                        dh, dw = tap // 3, tap % 3
                        mov = _ap_raw(xpad_ap,
                                      base_off + (c8 * 8 + dh) * 66 + dw,
                                      [(pstride, 128), (66, 8), (1, 64)])
                        nc.tensor.matmul(acc[:], pwt_b[:, tap, :], mov,
                                         start=(tap == 0), stop=(tap == 8))
                    nc.vector.tensor_copy(offs[:, c8 * 512:(c8 + 1) * 512],
                                          acc[:])
                for t in range(32):
                    tp = pcp.tile([128, 18], F32, tag="tpose")
                    nc.tensor.transpose(tp[:], offs[:, t * 128:(t + 1) * 128],
                                        id18[:])
                    nc.vector.tensor_copy(posT[:, t, :], tp[:])

            # ---------- positions / weights / indices ----------
            def ts(out, in_, s1, o1, s2=None, o2=None):
                if s2 is None:
                    nc.vector.tensor_scalar(out, in_, s1, None, op0=o1)
                else:
                    nc.vector.tensor_scalar(out, in_, s1, s2, op0=o1, op1=o2)

            _slab_ctr = [0]

            def slab():
                _slab_ctr[0] += 1
                return wpp.tile([128, 32, 2 * N], F32,
                                name=f"slab{_slab_ctr[0]}",
                                tag=f"slab{_slab_ctr[0]}")

            _wpp_cm = tc.tile_pool(name="wrapp", bufs=1)
            wpp = _wpp_cm.__enter__()
            p4 = base4                       # reuse base4 storage for p4
            nc.vector.tensor_add(p4[:], posT[:], base4[:])
            pc4 = slab()
            ts(pc4[:], p4[:], 4.0, OP.max, 67.0, OP.min)
            i32 = wpp.tile([128, 32, 2 * N], mybir.dt.int32)
            nc.vector.tensor_copy(i32[:], p4[:])
            mfr = slab()
            nc.vector.tensor_copy(mfr[:], i32[:])
            f4 = slab()
            nc.vector.tensor_tensor(f4[:], p4[:], mfr[:], op=OP.is_lt)
            nc.vector.tensor_tensor(f4[:], mfr[:], f4[:], op=OP.subtract)
            qlt = mfr                        # reuse
            ts(qlt[:], f4[:], 4.0, OP.max, 67.0, OP.min)
            qrb = slab()
            ts(qrb[:], f4[:], 1.0, OP.add, 4.0, OP.max)
            ts(qrb[:], qrb[:], 67.0, OP.min)
            g04 = slab()
            ts(g04[:], f4[:], 4.0, OP.max, 66.0, OP.min)
            ax = slab()
            nc.vector.tensor_tensor(ax[:], qlt[:], pc4[:], op=OP.subtract)
            ts(ax[:], ax[:], 1.0, OP.add)
            bx = qlt                         # reuse (qlt dead)
            nc.vector.tensor_tensor(bx[:], pc4[:], qrb[:], op=OP.subtract)
            ts(bx[:], bx[:], 1.0, OP.add)
            c1 = pc4                         # reuse (pc4 dead)
            ts(c1[:], f4[:], 66.0, OP.is_le)
            c2 = qrb                         # reuse (qrb dead)
            ts(c2[:], f4[:], 3.0, OP.is_le)
            w0 = f4                          # reuse (f4 dead)
            nc.vector.tensor_tensor(w0[:], ax[:], c1[:], op=OP.mult)
            t0 = slab()
            nc.vector.tensor_tensor(t0[:], bx[:], c2[:], op=OP.mult)
            nc.vector.tensor_add(w0[:], w0[:], t0[:])
            w1 = t0                          # reuse
            nc.vector.tensor_add(w1[:], ax[:], bx[:])
            nc.vector.tensor_tensor(w1[:], w1[:], w0[:], op=OP.subtract)

            # corner weights -> wslab[p, t, j, n] bf16; j: 0=x2 row order
            # (lt, lb, rt, rb) = (w0x*w0y, w1x*w0y, w0x*w1y, w1x*w1y)
            wslab = wpp.tile([128, 32, 4, 9], BF16)
            for j, (wx, wy) in enumerate([(w0, w0), (w1, w0), (w0, w1), (w1, w1)]):
                nc.vector.tensor_tensor(wslab[:, :, j, :], wx[:, :, 0:N],
                                        wy[:, :, N:2 * N], op=OP.mult)

            # idx = g04x*64 + g04y - 260 (4-space shift removal)
            idxf = ax                        # reuse (ax dead), use x-half
            ts(idxf[:, :, 0:N], g04[:, :, 0:N], 64.0, OP.mult, 260.0,
               OP.subtract)
            nc.vector.tensor_add(idxf[:, :, 0:N], idxf[:, :, 0:N],
                                 g04[:, :, N:2 * N])
            # route to k-linear DRAM order, k = 9*(128t + p) + n
            nc.sync.dma_start(
                _ap_raw(idx_dram[:], 0, [(9, 128), (1152, 32), (1, 9)]),
                idxf[:, :, 0:N])
            for j in range(4):
                nc.sync.dma_start(
                    _ap_raw(w_dram[:], j * NS,
                            [(9, 128), (1152, 32), (1, 9)]),
                    wslab[:, :, j, :])
            _wpp_cm.__exit__(None, None, None)

            # gather source AP over x2: overlapping 2-row windows
            x2_src = _ap_raw(x2_d[:], 0, [(2 * INC, S - 1), (1, 4 * INC)])

            # ---------- main loop: gather / combine / PE / y ----------
            y_raw0 = pp.tile([128, NS], BF16)
            chunk_ctr = [0]
            TOT_CHUNKS = NG * NCHUNK
            with tc.tile_pool(name="gather", bufs=2) as gp, \
                 tc.tile_pool(name="xo", bufs=2) as xop, \
                 tc.tile_pool(name="xcmp", bufs=3) as xcp, \
                 tc.tile_pool(name="psum_g", bufs=1, space="PSUM") as gpp, \
                 tc.tile_pool(name="psum_tp", bufs=2, space="PSUM") as tpp, \
                 tc.tile_pool(name="psum_y", bufs=2, space="PSUM") as typ:
                gm1 = gpp.tile([128, 129], F32, tag="gram")
                gpsum = gm1[:, 0:128]
                m1psum = gm1[:, 128:129]
                def emit_prep(gpr):
                    w4pair = gp.tile([128, 128], BF16, tag="w4pair")
                    wrd2 = gp.tile([128, 128], BF16, tag="wrd2")
                    for g2 in range(2):
                        for j in range(4):
                            nc.sync.dma_start(
                                wrd2[g2 * 64 + j * 16:g2 * 64 + j * 16 + 16, :],
                                _ap_raw(w_dram[:],
                                        j * NS + (2 * gpr + g2) * GH,
                                        [(128, 16), (1, 128)]))
                    wps2 = tpp.tile([128, 128], BF16, tag="wps2", bufs=1)
                    nc.tensor.transpose(wps2[:], wrd2[:], id128[:])
                    nc.scalar.activation(w4pair[:], wps2[:], AF.Copy)
                    idxws = []
                    for g2 in range(2):
                        g = 2 * gpr + g2
                        idxw = gp.tile([128, GH // 16], I16, tag="idxw",
                                       bufs=4)
                        idr = gp.tile([128, 16], F32, tag="idr", bufs=4)
                        nc.sync.dma_start(
                            idr[:],
                            _ap_raw(idx_dram[:], g * GH, [(16, 128), (1, 16)]))
                        idxT = tpp.tile([16, 128], F32, tag="idxT", bufs=1)
                        nc.tensor.transpose(idxT[:], idr[:], id128f[:])
                        ix16 = gp.tile([16, 128], I16, tag="ix16", bufs=4)
                        nc.vector.tensor_copy(ix16[:], idxT[:])
                        for v in range(8):
                            nc.sync.dma_start(idxw[16 * v:16 * (v + 1)],
                                              ix16[:])
                        idxws.append(idxw)
                    return w4pair, idxws

                def emit_pair(gpr, prep):
                    w4pair, idxws = prep
                    for g2 in range(2):
                        g = 2 * gpr + g2
                        dst = gp.tile([128, NCHUNK, 512], BF16, tag="gdst")
                        nc.gpsimd.dma_gather(
                            dst[:], x2_src, idxws[g2][:], GH, GH, 4 * INC,
                            elem_step=2 * INC, transpose=False,
                            single_packet=False)

                        # bilinear combine on DVE (stride-0 broadcast weights)
                        dd = dst[:]
                        doff, dstr = dd.offset, dd.ap[0][0]
                        ww = w4pair[:]
                        woff, wstr = ww.offset + g2 * 64, ww.ap[0][0]

                        def ds(j):
                            return _ap_raw(dd, doff + j * 128,
                                           [(dstr, 128), (512, NCHUNK),
                                            (1, 128)])

                        def wb(j):
                            return _ap_raw(ww, woff + j * 16,
                                           [(wstr, 128), (1, 16), (0, 128)])

                        xo = xop.tile([128, NCHUNK, 128], BF16, tag="xo")
                        nc.vector.tensor_tensor(ds(0), ds(0), wb(0), op=OP.mult)
                        nc.vector.tensor_tensor(ds(1), ds(1), wb(1), op=OP.mult)
                        nc.vector.tensor_tensor(ds(0), ds(0), ds(1), op=OP.add)
                        nc.vector.tensor_tensor(ds(2), ds(2), wb(2), op=OP.mult)
                        nc.vector.tensor_tensor(ds(3), ds(3), wb(3), op=OP.mult)
                        nc.vector.tensor_tensor(ds(2), ds(2), ds(3), op=OP.add)
                        nc.vector.tensor_tensor(xo[:], ds(0), ds(2), op=OP.add)

                        # per 4-chunk group: transpose+gram+m1, then y matmuls
                        for grp in range(NCHUNK // 4):
                            xcm = xcp.tile([128, 4, 128], BF16, tag="xcm")
                            for q in range(4):
                                i = grp * 4 + q
                                ci = chunk_ctr[0]
                                chunk_ctr[0] += 1
                                tp = tpp.tile([128, 128], BF16, tag="tp")
                                nc.tensor.transpose(tp[:], xo[:, i, :],
                                                    id128[:])
                                nc.tensor.matmul(gpsum, xo[:, i, :],
                                                 xo[:, i, :],
                                                 start=(ci == 0),
                                                 stop=(ci == TOT_CHUNKS - 1),
                                                 skip_group_check=True)
                                nc.tensor.matmul(m1psum, xo[:, i, :],
                                                 ones_b[:],
                                                 start=(ci == 0),
                                                 stop=(ci == TOT_CHUNKS - 1),
                                                 skip_group_check=True)
                                nc.scalar.activation(xcm[:, q, :], tp[:],
                                                     AF.Copy)
                            xmov = xcm[:].rearrange("p a b -> p (a b)")
                            col = g * GH + grp * 512
                            y0p = typ.tile([128, 512], F32, tag="y0")
                            nc.tensor.matmul(y0p[:], cwt_b[:, 0:128], xmov,
                                             start=True, stop=True)
                            y1p = typ.tile([128, 512], F32, tag="y1", bufs=1)
                            nc.tensor.matmul(y1p[:], cwt_b[:, 128:256], xmov,
                                             start=True, stop=True)
                            nc.scalar.activation(y_raw0[:, col:col + 512],
                                                 y0p[:], AF.Copy)
                            ystg = xcp.tile([128, 512], BF16, tag="ystg")
                            nc.vector.tensor_copy(ystg[:], y1p[:])
                            nc.sync.dma_start(y1_dram[:, col:col + 512],
                                              ystg[:])

                prep = emit_prep(0)
                for gpr in range(NG // 2):
                    nxt = emit_prep(gpr + 1) if gpr + 1 < NG // 2 else None
                    emit_pair(gpr, prep)
                    prep = nxt
                g_sb = wp.tile([128, 129], F32)
                nc.vector.tensor_copy(g_sb[:], gm1[:])

            # ---------- allreduce + BN coefficients ----------
            gsum = wp.tile([128, 129], F32)
            if stage >= 3:
                bounce_in = dp.tile([128, 129], F32)
                bounce_out = dp.tile([128, 129], F32, addr_space="Shared")
                nc.sync.dma_start(bounce_in[:], g_sb[:])
                nc.gpsimd.collective_compute(
                    "AllReduce", OP.add,
                    replica_groups=[list(range(N_CORES))],
                    ins=[bounce_in[:].opt()],
                    outs=[bounce_out[:].opt()])
                nc.sync.dma_start(gsum[:], bounce_out[:])
            else:
                nc.vector.tensor_scalar(gsum[:], g_sb[:], 8.0, None,
                                        op0=OP.mult)

            with tc.tile_pool(name="psum_s", bufs=1, space="PSUM") as sp:
                t1p = sp.tile([128, OUTC], F32)
                nc.tensor.matmul(t1p[:], gsum[:, 0:128], cwt[:],
                                 start=True, stop=True)
                m2 = wp.tile([128, OUTC], F32)
                nc.vector.tensor_tensor(m2[:], cwt[:], t1p[:], op=OP.mult)
                dvp = sp.tile([1, OUTC], F32)
                nc.tensor.matmul(dvp[:], ones_f[:], m2[:], start=True,
                                 stop=True)
                m1yp = sp.tile([1, OUTC], F32)
                nc.tensor.matmul(m1yp[:], gsum[:, 128:129], cwt[:],
                                 start=True, stop=True)

                meanv = wp.tile([1, OUTC], F32)
                ts(meanv[:], m1yp[:], 1.0 / S_TOT, OP.mult)
                varv = wp.tile([1, OUTC], F32)
                ts(varv[:], dvp[:], 1.0 / S_TOT, OP.mult)
                msq = wp.tile([1, OUTC], F32)
                nc.vector.tensor_tensor(msq[:], meanv[:], meanv[:], op=OP.mult)
                nc.vector.tensor_tensor(varv[:], varv[:], msq[:],
                                        op=OP.subtract)
                ts(varv[:], varv[:], EPS, OP.add)
                sd = wp.tile([1, OUTC], F32)
                nc.scalar.activation(sd[:], varv[:], AF.Sqrt)
                rsd = wp.tile([1, OUTC], F32)
                nc.vector.reciprocal(rsd[:], sd[:])
                a_v = wp.tile([1, OUTC], F32)
                nc.vector.tensor_tensor(a_v[:], rsd[:], gb[:, 0:OUTC],
                                        op=OP.mult)
                b_v = wp.tile([1, OUTC], F32)
                nc.vector.tensor_tensor(b_v[:], meanv[:], a_v[:], op=OP.mult)
                nc.vector.tensor_tensor(b_v[:], gb[:, OUTC:2 * OUTC], b_v[:],
                                        op=OP.subtract)

            nc.sync.dma_start(_ap_raw(ab_dram[:], 0, [(2, OUTC)]), a_v[:])
            nc.sync.dma_start(_ap_raw(ab_dram[:], 1, [(2, OUTC)]), b_v[:])
            ab = pp.tile([128, 2, 2], F32)
            nc.sync.dma_start(
                ab[:], _ap_raw(ab_dram[:], 0, [(2, 128), (256, 2), (1, 2)]))

            # ---------- silu epilogue ----------
            YB = 4096
            with tc.tile_pool(name="ybuf", bufs=3) as yb, \
                 tc.tile_pool(name="y1rd", bufs=3) as yr:
                y1t = []
                for blk in range(NS // YB):
                    y1blk = yr.tile([128, YB], BF16, tag="y1blk")
                    nc.sync.dma_start(y1blk[:],
                                      y1_dram[:, blk * YB:(blk + 1) * YB])
                    y1t.append(y1blk)
                    ybuf = yb.tile([128, YB], BF16, tag="yb")
                    nc.scalar.activation(
                        ybuf[:], y_raw0[:, blk * YB:(blk + 1) * YB],
                        AF.Silu, scale=ab[:, 0, 0:1], bias=ab[:, 0, 1:2])
                    out_ap = _ap_raw(
                        out_d[:], blk * YB,
                        [(NS, 128), (1, YB)])
                    nc.sync.dma_start(out_ap, ybuf[:])
                    ybuf1 = yb.tile([128, YB], BF16, tag="yb")
                    nc.scalar.activation(
                        ybuf1[:], y1blk[:],
                        AF.Silu, scale=ab[:, 1, 0:1], bias=ab[:, 1, 1:2])
                    out_ap1 = _ap_raw(
                        out_d[:], 128 * NS + blk * YB,
                        [(NS, 128), (1, YB)])
                    nc.sync.dma_start(out_ap1, ybuf1[:])

    nc.compile()
    return nc


def prep_inputs(x, pw, pb, cw, gamma, beta):
    x = np.asarray(x, np.float32)
    pw = np.asarray(pw, np.float32)
    pb = np.asarray(pb, np.float32)
    cw = np.asarray(cw, np.float32)
    gamma = np.asarray(gamma, np.float32)
    beta = np.asarray(beta, np.float32)

    pwt = np.ascontiguousarray(
        pw.reshape(2 * N, INC, 9).transpose(1, 2, 0))      # (128, 9, 18)

    angles = np.linspace(0.0, 2.0 * math.pi, N + 1, dtype=np.float64)[:-1]
    pn = np.concatenate([np.cos(angles), np.sin(angles)]).astype(np.float32)
    p_idx = np.arange(128)
    t_idx = np.arange(32)
    hh = (2 * t_idx[None, :] + (p_idx[:, None] >= 64)).astype(np.float32)
    ww = np.broadcast_to((p_idx % 64).astype(np.float32)[:, None], (128, 32))
    base4 = np.zeros((128, 32, 2 * N), np.float32)
    base4[:, :, 0:N] = hh[:, :, None] + (pn[0:N] + pb[0:N])[None, None, :] + 4.0
    base4[:, :, N:] = ww[:, :, None] + (pn[N:] + pb[N:])[None, None, :] + 4.0

    cwt = np.ascontiguousarray(cw[:, :, 0, 0].T)           # (128, 256)
    gb = np.concatenate([gamma, beta])[None, :]            # (1, 512)
    id128 = np.eye(128, dtype=np.float32).astype(ml_dtypes.bfloat16)

    in_maps = []
    for b in range(B):
        xb = x[b].reshape(INC, S)
        xpad = np.zeros((INC, 66, 66), ml_dtypes.bfloat16)
        xpad[:, 1:65, 1:65] = x[b]
        xT = np.ascontiguousarray(xb.T).astype(ml_dtypes.bfloat16)  # (4096, 128)
        x2 = np.zeros((S, 2 * INC), ml_dtypes.bfloat16)
        x2[:, 0:INC] = xT
        x2[:S - 64, INC:] = xT[64:]
        in_maps.append(dict(
            xpad=np.ascontiguousarray(xpad.reshape(INC, 66 * 66)), x2=x2,
            pwt=pwt, base4=base4, cwt=cwt, gb=gb,
            id18=np.eye(18, dtype=np.float32), id128=id128,
            id128f=np.eye(128, dtype=np.float32)))
    return in_maps


_NC_CACHE = {}


def kernel(x, pw, pb, cw, gamma, beta):
    import os
    if "nc" not in _NC_CACHE:
        _NC_CACHE["nc"] = build(
            new_idx=os.environ.get("NEWIDX", "1") == "1",
            new_w=os.environ.get("NEWW", "1") == "1")
    nc = _NC_CACHE["nc"]
    in_maps = prep_inputs(x, pw, pb, cw, gamma, beta)
    res = run_bass_kernel_spmd(nc, in_maps, core_ids=list(range(N_CORES)))
    out = np.stack([
        np.asarray(res.results[b]["out"]).astype(np.float32).reshape(
            OUTC, H, W * N)
        for b in range(B)])
    return out


# revision 17
# speedup vs baseline: 1.9659x; 1.3145x over previous
"""AKConv TRN2 kernel: 8-core data-parallel over batch.

Sample-major gather architecture: dma_gather(transpose=False) puts each
sample (4 corners x 128ch) on one partition; bilinear combine on DVE via
stride-0 broadcast weights; PE transposes chunks back to channel-major,
accumulates the BN gram + row sums off the same stationary, and runs the
1x1 conv for both outc halves inline. Tail: allreduce -> BN coeffs ->
SiLU -> output DMA.
"""
import sys
sys.path.insert(0, "/opt/trn_rl_repo")
import math
import numpy as np
import ml_dtypes
import bass_rust
import concourse.bass as bass
import concourse.tile as tile
from concourse import bacc, mybir
from concourse.bass_utils import run_bass_kernel_spmd

F32 = mybir.dt.float32
BF16 = mybir.dt.bfloat16
I16 = mybir.dt.int16
AF = mybir.ActivationFunctionType
OP = mybir.AluOpType

B, INC, H, W = 8, 128, 64, 64
OUTC, N = 256, 9
S = H * W                      # 4096 pixels per core
NS = N * S                     # 36864 samples per core
S_TOT = float(B * NS)          # BN sample count
EPS = 1e-5
N_CORES = 8

GH = 2048                      # samples per dma_gather
NG = NS // GH                  # 18 gathers
NCHUNK = GH // 128             # 16 chunks of 128 samples per gather


def _ap_raw(ap, offset, dims):
    a = ap.copy()
    a.offset = offset
    a.ap = bass_rust.VecI64Pair(dims)
    return a


def build(stage=3, new_idx=True, new_w=True):
    nc = bacc.Bacc("TRN2", target_bir_lowering=False, debug=False,
                   num_devices=N_CORES)
    xpad_d = nc.dram_tensor("xpad", [INC, 66 * 66], BF16, kind="ExternalInput")
    x2_d = nc.dram_tensor("x2", [S, 2 * INC], BF16, kind="ExternalInput")
    pwt_d = nc.dram_tensor("pwt", [INC, 9, 2 * N], F32, kind="ExternalInput")
    base4_d = nc.dram_tensor("base4", [128, 32, 2 * N], F32, kind="ExternalInput")
    cwt_d = nc.dram_tensor("cwt", [INC, OUTC], F32, kind="ExternalInput")
    gb_d = nc.dram_tensor("gb", [1, 2 * OUTC], F32, kind="ExternalInput")
    id18_d = nc.dram_tensor("id18", [18, 18], F32, kind="ExternalInput")
    id128_d = nc.dram_tensor("id128", [128, 128], BF16, kind="ExternalInput")
    id128f_d = nc.dram_tensor("id128f", [128, 128], F32, kind="ExternalInput")
    out_d = nc.dram_tensor("out", [OUTC, NS], BF16, kind="ExternalOutput")

    idx_dram = nc.dram_tensor("idx_scratch", [1, NS], F32, kind="Internal")
    w_dram = nc.dram_tensor("w_scratch", [4, NS], BF16, kind="Internal")
    ab_dram = nc.dram_tensor("ab_scratch", [OUTC, 2], F32, kind="Internal")
    y1_dram = nc.dram_tensor("y1_scratch", [128, NS], BF16, kind="Internal")

    with tile.TileContext(nc) as tc:
        with tc.tile_pool(name="persist", bufs=1) as pp, \
             tc.tile_pool(name="work", bufs=1) as wp, \
             tc.tile_pool(name="dram", bufs=1, space="DRAM") as dp:

            # ---------- loads ----------
            pwt = pp.tile([INC, 9, 2 * N], F32)
            nc.sync.dma_start(pwt[:], pwt_d[:])
            pwt_b = pp.tile([INC, 9, 2 * N], BF16)
            nc.vector.tensor_copy(pwt_b[:], pwt[:])
            base4 = pp.tile([128, 32, 2 * N], F32)
            nc.sync.dma_start(base4[:], base4_d[:])
            cwt = pp.tile([INC, OUTC], F32)
            nc.sync.dma_start(cwt[:], cwt_d[:])
            gb = pp.tile([1, 2 * OUTC], F32)
            nc.sync.dma_start(gb[:], gb_d[:])
            id128 = pp.tile([128, 128], BF16)
            nc.sync.dma_start(id128[:], id128_d[:])
            id128f = pp.tile([128, 128], F32)
            nc.sync.dma_start(id128f[:], id128f_d[:])
            cwt_b = pp.tile([INC, OUTC], BF16)
            nc.vector.tensor_copy(cwt_b[:], cwt[:])
            ones_b = pp.tile([128, 1], BF16)
            nc.vector.memset(ones_b[:], 1.0)
            ones_f = pp.tile([128, 1], F32)
            nc.vector.memset(ones_f[:], 1.0)
            warm_in = dp.tile([1, 4], F32)
            warm_out = dp.tile([1, 4], F32, addr_space="Shared")
            if stage >= 3:
                nc.sync.dma_start(warm_in[:], gb_d[:, 0:4])
                nc.gpsimd.collective_compute(
                    "AllReduce", OP.add,
                    replica_groups=[list(range(N_CORES))],
                    ins=[warm_in[:].opt()],
                    outs=[warm_out[:].opt()])
            posT = pp.tile([128, 32, 2 * N], F32)

            # ---------- p_conv: offsets (18, S) then transpose ----------
            id18 = pp.tile([18, 18], F32)
            nc.sync.dma_start(id18[:], id18_d[:])
            with tc.tile_pool(name="xpadp", bufs=1) as xp, \
                 tc.tile_pool(name="psum_pc", bufs=4, space="PSUM") as pcp:
                xpad = xp.tile([INC, 66 * 66], BF16)
                nc.sync.dma_start(xpad[:], xpad_d[:])
                xpad_ap = xpad[:]
                pstride = xpad_ap.ap[0][0]
                base_off = xpad_ap.offset
                offs = xp.tile([18, S], F32)
                for c8 in range(8):
                    acc = pcp.tile([18, 512], F32, tag="pconv")
                    for tap in range(9):
                        dh, dw = tap // 3, tap % 3
                        mov = _ap_raw(xpad_ap,
                                      base_off + (c8 * 8 + dh) * 66 + dw,
                                      [(pstride, 128), (66, 8), (1, 64)])
                        nc.tensor.matmul(acc[:], pwt_b[:, tap, :], mov,
                                         start=(tap == 0), stop=(tap == 8))
                    nc.vector.tensor_copy(offs[:, c8 * 512:(c8 + 1) * 512],
                                          acc[:])
                for t in range(32):
                    tp = pcp.tile([128, 18], F32, tag="tpose")
                    nc.tensor.transpose(tp[:], offs[:, t * 128:(t + 1) * 128],
                                        id18[:])
                    nc.vector.tensor_copy(posT[:, t, :], tp[:])

            # ---------- positions / weights / indices ----------
            def ts(out, in_, s1, o1, s2=None, o2=None):
                if s2 is None:
                    nc.vector.tensor_scalar(out, in_, s1, None, op0=o1)
                else:
                    nc.vector.tensor_scalar(out, in_, s1, s2, op0=o1, op1=o2)

            _slab_ctr = [0]

            def slab():
                _slab_ctr[0] += 1
                return wpp.tile([128, 32, 2 * N], F32,
                                name=f"slab{_slab_ctr[0]}",
                                tag=f"slab{_slab_ctr[0]}")

            _wpp_cm = tc.tile_pool(name="wrapp", bufs=1)
            wpp = _wpp_cm.__enter__()
            p4 = base4                       # reuse base4 storage for p4
            nc.vector.tensor_add(p4[:], posT[:], base4[:])
            pc4 = slab()
            ts(pc4[:], p4[:], 4.0, OP.max, 67.0, OP.min)
            i32 = wpp.tile([128, 32, 2 * N], mybir.dt.int32)
            nc.vector.tensor_copy(i32[:], p4[:])
            mfr = slab()
            nc.vector.tensor_copy(mfr[:], i32[:])
            f4 = slab()
            nc.vector.tensor_tensor(f4[:], p4[:], mfr[:], op=OP.is_lt)
            nc.vector.tensor_tensor(f4[:], mfr[:], f4[:], op=OP.subtract)
            qlt = mfr                        # reuse
            ts(qlt[:], f4[:], 4.0, OP.max, 67.0, OP.min)
            qrb = slab()
            ts(qrb[:], f4[:], 1.0, OP.add, 4.0, OP.max)
            ts(qrb[:], qrb[:], 67.0, OP.min)
            g04 = slab()
            ts(g04[:], f4[:], 4.0, OP.max, 66.0, OP.min)
            ax = slab()
            nc.vector.tensor_tensor(ax[:], qlt[:], pc4[:], op=OP.subtract)
            ts(ax[:], ax[:], 1.0, OP.add)
            bx = qlt                         # reuse (qlt dead)
            nc.vector.tensor_tensor(bx[:], pc4[:], qrb[:], op=OP.subtract)
            ts(bx[:], bx[:], 1.0, OP.add)
            c1 = pc4                         # reuse (pc4 dead)
            ts(c1[:], f4[:], 66.0, OP.is_le)
            c2 = qrb                         # reuse (qrb dead)
            ts(c2[:], f4[:], 3.0, OP.is_le)
            w0 = f4                          # reuse (f4 dead)
            nc.vector.tensor_tensor(w0[:], ax[:], c1[:], op=OP.mult)
            t0 = slab()
            nc.vector.tensor_tensor(t0[:], bx[:], c2[:], op=OP.mult)
            nc.vector.tensor_add(w0[:], w0[:], t0[:])
            w1 = t0                          # reuse
            nc.vector.tensor_add(w1[:], ax[:], bx[:])
            nc.vector.tensor_tensor(w1[:], w1[:], w0[:], op=OP.subtract)

            # corner weights -> wslab[p, t, j, n] bf16; j: 0=x2 row order
            # (lt, lb, rt, rb) = (w0x*w0y, w1x*w0y, w0x*w1y, w1x*w1y)
            wslab = wpp.tile([128, 32, 4, 9], BF16)
            for j, (wx, wy) in enumerate([(w0, w0), (w1, w0), (w0, w1), (w1, w1)]):
                nc.vector.tensor_tensor(wslab[:, :, j, :], wx[:, :, 0:N],
                                        wy[:, :, N:2 * N], op=OP.mult)

            # idx = g04x*64 + g04y - 260 (4-space shift removal)
            idxf = ax                        # reuse (ax dead), use x-half
            ts(idxf[:, :, 0:N], g04[:, :, 0:N], 64.0, OP.mult, 260.0,
               OP.subtract)
            nc.vector.tensor_add(idxf[:, :, 0:N], idxf[:, :, 0:N],
                                 g04[:, :, N:2 * N])
            # route to k-linear DRAM order, k = 9*(128t + p) + n.
            # PE-transpose slabs to t-partition layout first so the DRAM
            # writes are fully contiguous (32 descriptors instead of ~20k).
            wT2 = wpp.tile([32, 4, 128, 9], BF16)
            iT2 = wpp.tile([32, 128, 9], F32)
            with tc.tile_pool(name="psum_wr", bufs=2, space="PSUM") as pwr:
                for n_ in range(9):
                    tpi = pwr.tile([32, 128], F32, tag="tpi")
                    nc.tensor.transpose(tpi[:], idxf[:, :, n_], id128f[:])
                    nc.scalar.activation(iT2[:, :, n_], tpi[:], AF.Copy)
                    for j in range(4):
                        tpw = pwr.tile([32, 128], BF16, tag="tpw")
                        nc.tensor.transpose(tpw[:], wslab[:, :, j, n_],
                                            id128[:])
                        nc.scalar.activation(wT2[:, j, :, n_], tpw[:],
                                             AF.Copy)
            nc.sync.dma_start(
                _ap_raw(idx_dram[:], 0, [(1152, 32), (9, 128), (1, 9)]),
                iT2[:])
            for j in range(4):
                nc.sync.dma_start(
                    _ap_raw(w_dram[:], j * NS,
                            [(1152, 32), (9, 128), (1, 9)]),
                    wT2[:, j, :, :])
            _wpp_cm.__exit__(None, None, None)

            # gather source AP over x2: overlapping 2-row windows
            x2_src = _ap_raw(x2_d[:], 0, [(2 * INC, S - 1), (1, 4 * INC)])

            # ---------- main loop: gather / combine / PE / y ----------
            y_raw0 = pp.tile([128, NS], BF16)
            chunk_ctr = [0]
            TOT_CHUNKS = NG * NCHUNK
            with tc.tile_pool(name="gather", bufs=2) as gp, \
                 tc.tile_pool(name="xo", bufs=2) as xop, \
                 tc.tile_pool(name="xcmp", bufs=3) as xcp, \
                 tc.tile_pool(name="psum_g", bufs=1, space="PSUM") as gpp, \
                 tc.tile_pool(name="psum_tp", bufs=2, space="PSUM") as tpp, \
                 tc.tile_pool(name="psum_y", bufs=2, space="PSUM") as typ:
                gm1 = gpp.tile([128, 129], F32, tag="gram")
                gpsum = gm1[:, 0:128]
                m1psum = gm1[:, 128:129]
                def emit_prep(gpr):
                    w4pair = gp.tile([128, 128], BF16, tag="w4pair")
                    wrd2 = gp.tile([128, 128], BF16, tag="wrd2")
                    for g2 in range(2):
                        for j in range(4):
                            nc.sync.dma_start(
                                wrd2[g2 * 64 + j * 16:g2 * 64 + j * 16 + 16, :],
                                _ap_raw(w_dram[:],
                                        j * NS + (2 * gpr + g2) * GH,
                                        [(128, 16), (1, 128)]))
                    wps2 = tpp.tile([128, 128], BF16, tag="wps2", bufs=1)
                    nc.tensor.transpose(wps2[:], wrd2[:], id128[:])
                    nc.scalar.activation(w4pair[:], wps2[:], AF.Copy)
                    idxws = []
                    for g2 in range(2):
                        g = 2 * gpr + g2
                        idxw = gp.tile([128, GH // 16], I16, tag="idxw",
                                       bufs=4)
                        idr = gp.tile([128, 16], F32, tag="idr", bufs=4)
                        nc.sync.dma_start(
                            idr[:],
                            _ap_raw(idx_dram[:], g * GH, [(16, 128), (1, 16)]))
                        idxT = tpp.tile([16, 128], F32, tag="idxT", bufs=1)
                        nc.tensor.transpose(idxT[:], idr[:], id128f[:])
                        ix16 = gp.tile([16, 128], I16, tag="ix16", bufs=4)
                        nc.vector.tensor_copy(ix16[:], idxT[:])
                        for v in range(8):
                            nc.sync.dma_start(idxw[16 * v:16 * (v + 1)],
                                              ix16[:])
                        idxws.append(idxw)
                    return w4pair, idxws

                def emit_pair(gpr, prep):
                    w4pair, idxws = prep
                    xos = []
                    for g2 in range(2):
                        g = 2 * gpr + g2
                        dst = gp.tile([128, NCHUNK, 512], BF16, tag="gdst", bufs=3)
                        nc.gpsimd.dma_gather(
                            dst[:], x2_src, idxws[g2][:], GH, GH, 4 * INC,
                            elem_step=2 * INC, transpose=False,
                            single_packet=False)

                        # bilinear combine on DVE (stride-0 broadcast weights)
                        dd = dst[:]
                        doff, dstr = dd.offset, dd.ap[0][0]
                        ww = w4pair[:]
                        woff, wstr = ww.offset + g2 * 64, ww.ap[0][0]

                        def ds(j):
                            return _ap_raw(dd, doff + j * 128,
                                           [(dstr, 128), (512, NCHUNK),
                                            (1, 128)])

                        def wb(j):
                            return _ap_raw(ww, woff + j * 16,
                                           [(wstr, 128), (1, 16), (0, 128)])

                        xo = xop.tile([128, NCHUNK, 128], BF16, tag="xo", bufs=3)
                        nc.vector.tensor_tensor(ds(0), ds(0), wb(0), op=OP.mult)
                        nc.vector.tensor_tensor(ds(1), ds(1), wb(1), op=OP.mult)
                        nc.vector.tensor_tensor(ds(0), ds(0), ds(1), op=OP.add)
                        nc.vector.tensor_tensor(ds(2), ds(2), wb(2), op=OP.mult)
                        nc.vector.tensor_tensor(ds(3), ds(3), wb(3), op=OP.mult)
                        nc.vector.tensor_tensor(ds(2), ds(2), ds(3), op=OP.add)
                        nc.vector.tensor_tensor(xo[:], ds(0), ds(2), op=OP.add)
                        xos.append(xo)

                    for g2 in range(2):
                        g = 2 * gpr + g2
                        xo = xos[g2]
                        # per 4-chunk group: transpose+gram+m1, then y matmuls
                        for grp in range(NCHUNK // 4):
                            xcm = xcp.tile([128, 4, 128], BF16, tag="xcm")
                            for q in range(4):
                                i = grp * 4 + q
                                ci = chunk_ctr[0]
                                chunk_ctr[0] += 1
                                tp = tpp.tile([128, 128], BF16, tag="tp")
                                nc.tensor.transpose(tp[:], xo[:, i, :],
                                                    id128[:])
                                nc.tensor.matmul(gpsum, xo[:, i, :],
                                                 xo[:, i, :],
                                                 start=(ci == 0),
                                                 stop=(ci == TOT_CHUNKS - 1),
                                                 skip_group_check=True)
                                nc.tensor.matmul(m1psum, xo[:, i, :],
                                                 ones_b[:],
                                                 start=(ci == 0),
                                                 stop=(ci == TOT_CHUNKS - 1),
                                                 skip_group_check=True)
                                nc.scalar.activation(xcm[:, q, :], tp[:],
                                                     AF.Copy)
                            xmov = xcm[:].rearrange("p a b -> p (a b)")
                            col = g * GH + grp * 512
                            y0p = typ.tile([128, 512], F32, tag="y0")
                            nc.tensor.matmul(y0p[:], cwt_b[:, 0:128], xmov,
                                             start=True, stop=True)
                            y1p = typ.tile([128, 512], F32, tag="y1", bufs=1)
                            nc.tensor.matmul(y1p[:], cwt_b[:, 128:256], xmov,
                                             start=True, stop=True)
                            nc.scalar.activation(y_raw0[:, col:col + 512],
                                                 y0p[:], AF.Copy)
                            ystg = xcp.tile([128, 512], BF16, tag="ystg")
                            nc.scalar.activation(ystg[:], y1p[:], AF.Copy)
                            nc.sync.dma_start(y1_dram[:, col:col + 512],
                                              ystg[:])

                prep = emit_prep(0)
                for gpr in range(NG // 2):
                    nxt = emit_prep(gpr + 1) if gpr + 1 < NG // 2 else None
                    emit_pair(gpr, prep)
                    prep = nxt
                g_sb = wp.tile([128, 129], F32)
                nc.vector.tensor_copy(g_sb[:], gm1[:])

            # ---------- allreduce + BN coefficients ----------
            gsum = wp.tile([128, 129], F32)
            if stage >= 3:
                bounce_in = dp.tile([128, 129], F32)
                bounce_out = dp.tile([128, 129], F32, addr_space="Shared")
                nc.sync.dma_start(bounce_in[:], g_sb[:])
                nc.gpsimd.collective_compute(
                    "AllReduce", OP.add,
                    replica_groups=[list(range(N_CORES))],
                    ins=[bounce_in[:].opt()],
                    outs=[bounce_out[:].opt()])
                nc.sync.dma_start(gsum[:], bounce_out[:])
            else:
                nc.vector.tensor_scalar(gsum[:], g_sb[:], 8.0, None,
                                        op0=OP.mult)

            with tc.tile_pool(name="psum_s", bufs=1, space="PSUM") as sp:
                t1p = sp.tile([128, OUTC], F32)
                nc.tensor.matmul(t1p[:], gsum[:, 0:128], cwt[:],
                                 start=True, stop=True)
                m2 = wp.tile([128, OUTC], F32)
                nc.vector.tensor_tensor(m2[:], cwt[:], t1p[:], op=OP.mult)
                dvp = sp.tile([1, OUTC], F32)
                nc.tensor.matmul(dvp[:], ones_f[:], m2[:], start=True,
                                 stop=True)
                m1yp = sp.tile([1, OUTC], F32)
                nc.tensor.matmul(m1yp[:], gsum[:, 128:129], cwt[:],
                                 start=True, stop=True)

                meanv = wp.tile([1, OUTC], F32)
                ts(meanv[:], m1yp[:], 1.0 / S_TOT, OP.mult)
                varv = wp.tile([1, OUTC], F32)
                ts(varv[:], dvp[:], 1.0 / S_TOT, OP.mult)
                msq = wp.tile([1, OUTC], F32)
                nc.vector.tensor_tensor(msq[:], meanv[:], meanv[:], op=OP.mult)
                nc.vector.tensor_tensor(varv[:], varv[:], msq[:],
                                        op=OP.subtract)
                ts(varv[:], varv[:], EPS, OP.add)
                sd = wp.tile([1, OUTC], F32)
                nc.scalar.activation(sd[:], varv[:], AF.Sqrt)
                rsd = wp.tile([1, OUTC], F32)
                nc.vector.reciprocal(rsd[:], sd[:])
                a_v = wp.tile([1, OUTC], F32)
                nc.vector.tensor_tensor(a_v[:], rsd[:], gb[:, 0:OUTC],
                                        op=OP.mult)
                b_v = wp.tile([1, OUTC], F32)
                nc.vector.tensor_tensor(b_v[:], meanv[:], a_v[:], op=OP.mult)
                nc.vector.tensor_tensor(b_v[:], gb[:, OUTC:2 * OUTC], b_v[:],
                                        op=OP.subtract)

            nc.sync.dma_start(_ap_raw(ab_dram[:], 0, [(2, OUTC)]), a_v[:])
            nc.sync.dma_start(_ap_raw(ab_dram[:], 1, [(2, OUTC)]), b_v[:])
            ab = pp.tile([128, 2, 2], F32)
            nc.sync.dma_start(
                ab[:], _ap_raw(ab_dram[:], 0, [(2, 128), (256, 2), (1, 2)]))

            # ---------- silu epilogue ----------
            YB = 4096
            with tc.tile_pool(name="ybuf", bufs=3) as yb, \
                 tc.tile_pool(name="y1rd", bufs=3) as yr:
                y1t = []
                for blk in range(NS // YB):
                    y1blk = yr.tile([128, YB], BF16, tag="y1blk")
                    nc.sync.dma_start(y1blk[:],
                                      y1_dram[:, blk * YB:(blk + 1) * YB])
                    y1t.append(y1blk)
                    ybuf = yb.tile([128, YB], BF16, tag="yb")
                    nc.scalar.activation(
                        ybuf[:], y_raw0[:, blk * YB:(blk + 1) * YB],
                        AF.Silu, scale=ab[:, 0, 0:1], bias=ab[:, 0, 1:2])
                    out_ap = _ap_raw(
                        out_d[:], blk * YB,
                        [(NS, 128), (1, YB)])
                    nc.sync.dma_start(out_ap, ybuf[:])
                    ybuf1 = yb.tile([128, YB], BF16, tag="yb")
                    nc.scalar.activation(
                        ybuf1[:], y1blk[:],
                        AF.Silu, scale=ab[:, 1, 0:1], bias=ab[:, 1, 1:2])
                    out_ap1 = _ap_raw(
                        out_d[:], 128 * NS + blk * YB,
                        [(NS, 128), (1, YB)])
                    nc.sync.dma_start(out_ap1, ybuf1[:])

    nc.compile()
    return nc


def prep_inputs(x, pw, pb, cw, gamma, beta):
    x = np.asarray(x, np.float32)
    pw = np.asarray(pw, np.float32)
    pb = np.asarray(pb, np.float32)
    cw = np.asarray(cw, np.float32)
    gamma = np.asarray(gamma, np.float32)
    beta = np.asarray(beta, np.float32)

    pwt = np.ascontiguousarray(
        pw.reshape(2 * N, INC, 9).transpose(1, 2, 0))      # (128, 9, 18)

    angles = np.linspace(0.0, 2.0 * math.pi, N + 1, dtype=np.float64)[:-1]
    pn = np.concatenate([np.cos(angles), np.sin(angles)]).astype(np.float32)
    p_idx = np.arange(128)
    t_idx = np.arange(32)
    hh = (2 * t_idx[None, :] + (p_idx[:, None] >= 64)).astype(np.float32)
    ww = np.broadcast_to((p_idx % 64).astype(np.float32)[:, None], (128, 32))
    base4 = np.zeros((128, 32, 2 * N), np.float32)
    base4[:, :, 0:N] = hh[:, :, None] + (pn[0:N] + pb[0:N])[None, None, :] + 4.0
    base4[:, :, N:] = ww[:, :, None] + (pn[N:] + pb[N:])[None, None, :] + 4.0

    cwt = np.ascontiguousarray(cw[:, :, 0, 0].T)           # (128, 256)
    gb = np.concatenate([gamma, beta])[None, :]            # (1, 512)
    id128 = np.eye(128, dtype=np.float32).astype(ml_dtypes.bfloat16)

    in_maps = []
    for b in range(B):
        xb = x[b].reshape(INC, S)
        xpad = np.zeros((INC, 66, 66), ml_dtypes.bfloat16)
        xpad[:, 1:65, 1:65] = x[b]
        xT = np.ascontiguousarray(xb.T).astype(ml_dtypes.bfloat16)  # (4096, 128)
        x2 = np.zeros((S, 2 * INC), ml_dtypes.bfloat16)
        x2[:, 0:INC] = xT
        x2[:S - 64, INC:] = xT[64:]
        in_maps.append(dict(
            xpad=np.ascontiguousarray(xpad.reshape(INC, 66 * 66)), x2=x2,
            pwt=pwt, base4=base4, cwt=cwt, gb=gb,
            id18=np.eye(18, dtype=np.float32), id128=id128,
            id128f=np.eye(128, dtype=np.float32)))
    return in_maps


_NC_CACHE = {}


def kernel(x, pw, pb, cw, gamma, beta):
    import os
    if "nc" not in _NC_CACHE:
        _NC_CACHE["nc"] = build(
            new_idx=os.environ.get("NEWIDX", "1") == "1",
            new_w=os.environ.get("NEWW", "1") == "1")
    nc = _NC_CACHE["nc"]
    in_maps = prep_inputs(x, pw, pb, cw, gamma, beta)
    res = run_bass_kernel_spmd(nc, in_maps, core_ids=list(range(N_CORES)))
    out = np.stack([
        np.asarray(res.results[b]["out"]).astype(np.float32).reshape(
            OUTC, H, W * N)
        for b in range(B)])
    return out
